# revision 25
# baseline (speedup 1.0000x reference)
"""Multi-head attention kernel for Trainium2, 8 NeuronCores.

Problem (hardcoded shapes): B=4, S=2048, E=1024, H=16, DH=64.
  q/k/v = einsum('bse,hed->bhsd', x, W{q,k,v}) + b{q,k,v}
  attn  = softmax(q k^T / sqrt(DH)) v
  out   = concat_heads(attn) @ Wo^T + bo

Sharding: core c -> (batch b = c//2, head-half hh = c%2, i.e. heads
8*hh..8*hh+7).  Each core computes a [S, E] partial of its batch's output;
the host sums the two partials per batch and adds bo.

Everything 2-byte (fp16) on the PE so every matmul runs at 1 col/cycle and
the DMA xbar can do the transposes:
  xT   [e=128 x 8, s=2048]  f16, via dma_start_transpose (no PE/DVE cost)
  qT/kT [j=128, pair, s]    f16, Wq^T x + bias per pair (PE + DVE bias)
  vext [t, tb, h, 65]       f16, x Wv + bias, with a fused ones column
  scores [t=128, s=1024]    PSUM f32, one matmul per (h, s-chunk, t-block)
  exp   [t=128, s=1024]     ACT Exp(scale=1/8) -> SBUF f16
  attnV FLIPPED: out [s=128, 65] += exp-block^T (lhsT) @ vext[t,65] (rhs);
        65-col moving operand -> half the PE cost of the [65, s] orientation
  normalize: DVE recip on the sums column + per-partition tensor_scalar
  concatT via dma_start_transpose of the normalized [s, f] blocks
  outproj: per-pair partial matmuls accumulated in SBUF f32 by DVE, so the
        last pair's work is the only tail
"""

import os
import sys

for _p in ("/opt/trn_rl_repo", "/root/.axon_site/_ro/trn_rl_repo"):
    if os.path.isdir(_p) and _p not in sys.path:
        sys.path.insert(0, _p)
        break

from collections import deque
from contextlib import ExitStack

import numpy as np

import concourse.bass as bass
import concourse.tile as tile
import concourse.mybir as mybir
from concourse import bacc, bass_utils

B, S, E, H, DH = 4, 2048, 1024, 16, 64
HPC = 8           # heads per core
JW = HPC * DH     # 512, per-core qkv width
N_CORES = 8
EB = E // 128     # 8 e-blocks
TB = S // 128     # 16 t-blocks
SB8 = 8           # s-blocks per 1024-wide s-chunk
F32 = mybir.dt.float32
F16 = mybir.dt.float16
Exp = mybir.ActivationFunctionType.Exp
ADD = mybir.AluOpType.add

# (sc, h) processing order: interleave the two s-chunks so each qk pair's
# projection window is ~4 units wide and outproj partials spread out.
UNITS = []
for hp in range(4):
    UNITS += [(0, 2 * hp), (0, 2 * hp + 1), (1, 2 * hp), (1, 2 * hp + 1)]


def _emit(tc, aps, ctx, dbg=None):
    nc = tc.nc
    x_d, wq_d, wk_d, wv_d, wo_d, bqt_d, bkt_d, bv_d, out_d = aps

    def pool(**kw):
        return ctx.enter_context(tc.tile_pool(**kw))

    const = pool(name="const", bufs=1)
    xTp = pool(name="xT", bufs=1)
    vxp = pool(name="vext", bufs=1)
    wqk = pool(name="wqk", bufs=2)
    qkp = pool(name="qk", bufs=1)
    exp_p = pool(name="expS", bufs=20)
    st2p = pool(name="st2", bufs=2)
    recp = pool(name="rec", bufs=2)
    ccp = pool(name="concatT", bufs=1)
    outp = pool(name="outs", bufs=2)
    ps_slot = pool(name="ps_slot", bufs=2, space="PSUM")
    ps_acc = pool(name="ps_acc", bufs=1, space="PSUM")
    ps_proj = pool(name="ps_proj", bufs=1, space="PSUM")

    # ---- pair-0 weights first on SP, then the x transposes split across the
    # two HWDGE engines so they all issue within ~3us ----
    wq_r = wq_d.rearrange("(eb pp) j -> pp eb j", pp=128)
    wk_r = wk_d.rearrange("(eb pp) j -> pp eb j", pp=128)

    # NOTE: the DMA-transpose xbar is a single shared unit — concurrent
    # transposes issued from different HWDGE queues corrupt each other
    # (verified empirically).  ALL dma_start_transpose go through nc.sync;
    # ordinary weight DMAs go through nc.scalar so they overlap.
    def load_pair_weights(p, eng=None):
        eng = eng or nc.sync
        wq_t = wqk.tile([128, EB, 128], F16, tag="wq", name=f"wq{p}")
        eng.dma_start(wq_t[:], wq_r[:, :, p * 128:(p + 1) * 128])
        wk_t = wqk.tile([128, EB, 128], F16, tag="wk", name=f"wk{p}")
        eng.dma_start(wk_t[:], wk_r[:, :, p * 128:(p + 1) * 128])
        return wq_t, wk_t

    pair_w = {0: load_pair_weights(0, eng=nc.scalar)}

    xT = xTp.tile([128, EB, S], F16)
    for eb in range(EB):
        nc.sync.dma_start_transpose(xT[:, eb, :], x_d[:, eb * 128:(eb + 1) * 128])

    wv_sb = const.tile([128, EB, JW], F16)
    nc.scalar.dma_start(wv_sb[:], wv_d.rearrange("(eb p) j -> p eb j", p=128))
    wo_sb = const.tile([128, 4, E], F16)
    nc.scalar.dma_start(wo_sb[:], wo_d.rearrange("(fb p) e -> p fb e", p=128))
    bq_sb = const.tile([128, 4], F32)
    nc.scalar.dma_start(bq_sb[:], bqt_d[:])
    bk_sb = const.tile([128, 4], F32)
    nc.scalar.dma_start(bk_sb[:], bkt_d[:])
    bv1 = const.tile([1, JW], F32)
    nc.scalar.dma_start(bv1[:], bv_d[:])
    bvb = const.tile([128, JW], F32)
    nc.gpsimd.partition_broadcast(bvb[:], bv1[:])

    vext = vxp.tile([128, TB, HPC, DH + 1], F16)
    nc.gpsimd.memset(vext[:, :, :, DH:DH + 1], 1.0)

    # qT/kT for all 4 pairs stay resident (f16, 16KB/partition total)
    qT = qkp.tile([128, 4, S], F16, tag="qT")
    kT = qkp.tile([128, 4, S], F16, tag="kT")

    # ---- injectable PE work chunks ----
    def qk_chunk(p, qk, c):
        """One 512-col q/k projection chunk (8 e-block matmuls + bias)."""
        def emit():
            wq_t, wk_t = pair_w[p]
            w_t, dst, b_sb = ((wq_t, qT, bq_sb) if qk == 0 else
                              (wk_t, kT, bk_sb))
            pq = ps_proj.tile([128, 512], F32, tag="pq",
                              name=f"pq_{p}_{qk}_{c}")
            for eb in range(EB):
                nc.tensor.matmul(pq[:], w_t[:, eb, :],
                                 xT[:, eb, c * 512:(c + 1) * 512],
                                 start=(eb == 0), stop=(eb == EB - 1))
            nc.vector.tensor_scalar_add(
                dst[:, p, c * 512:(c + 1) * 512], pq[:], b_sb[:, p:p + 1])
        return emit

    def v_chunk(tb, hp):
        """Project v for t-block tb, head pair hp (2 heads, 128 cols)."""
        def emit():
            pv = ps_proj.tile([128, 128], F32, tag="proj",
                              name=f"pv_{tb}_{hp}")
            for eb in range(EB):
                nc.tensor.matmul(pv[:], xT[:, eb, tb * 128:(tb + 1) * 128],
                                 wv_sb[:, eb, hp * 128:(hp + 1) * 128],
                                 start=(eb == 0), stop=(eb == EB - 1))
            nc.vector.tensor_tensor(
                vext[:, tb, 2 * hp:2 * hp + 2, 0:DH],
                pv[:].rearrange("p (h d) -> p h d", h=2),
                bvb[:, hp * 128:(hp + 1) * 128].rearrange(
                    "p (h d) -> p h d", h=2), ADD)
        return emit

    concatT = ccp.tile([128, 4, S], F16)
    oacc = {}  # sc -> SBUF f32 accumulator

    def outproj_partial(sc, sb8, ec, p):
        def emit():
            po = ps_proj.tile([128, 512], F32, tag="proj",
                              name=f"po_{sc}_{sb8}_{ec}_{p}")
            nc.tensor.matmul(po[:],
                             concatT[:, p, sc * 1024 + sb8 * 128:
                                     sc * 1024 + (sb8 + 1) * 128],
                             wo_sb[:, p, ec * 512:(ec + 1) * 512],
                             start=True, stop=True)
            oa = oacc[sc][:, sb8, ec, :]
            if p == 0:
                nc.vector.tensor_copy(oa, po[:])
            else:
                nc.vector.tensor_tensor(oa, po[:], oa, ADD)
            if p == 3:
                r0 = sc * 1024 + sb8 * 128
                nc.sync.dma_start(out_d[r0:r0 + 128, ec * 512:(ec + 1) * 512],
                                  oacc[sc][:, sb8, ec, :])
        return emit

    # k chunk 0 / q chunks 0,1 of pair 0 and v t-block 0 are needed by the
    # very first attention iterations: emit them in the prefix.
    qk_chunk(0, 1, 0)()
    qk_chunk(0, 0, 0)()
    qk_chunk(0, 0, 1)()
    v_chunk(0, 0)()

    # ---- build the due-scheduled injection queue ----
    inj = []   # (due_iter, fn) pre-sorted
    injl = deque()  # runtime-enqueued (outproj partials), FIFO

    # pair0 remaining chunks (k s4-1..3 urgent, q s4-2,3 by iter 32)
    for c, due in ((1, 2), (2, 6), (3, 10)):
        inj.append((due, qk_chunk(0, 1, c)))
    for c, due in ((2, 16), (3, 20)):
        inj.append((due, qk_chunk(0, 0, c)))
    # v chunks: pair hp needed from unit 4*hp (iter 64*hp), h0/h1 from start
    for tb in range(1, TB):
        inj.append((min(2 * tb, 12 + tb // 2), v_chunk(tb, 0)))
    for hp, base in ((1, 40), (2, 104), (3, 168)):
        for tb in range(TB):
            inj.append((base + tb, v_chunk(tb, hp)))
    # pairs 1..3: weights load + 8 projection chunks each
    for p, base in ((1, 24), (2, 86), (3, 150)):
        def mk_load(p=p):
            def emit():
                pair_w[p] = load_pair_weights(p)
            return emit
        inj.append((base - 2, mk_load()))
        seq = [(1, 0), (0, 0), (0, 1), (1, 1), (1, 2), (1, 3)]
        for i, (qk, c) in enumerate(seq):
            inj.append((base + 4 * i, qk_chunk(p, qk, c)))
        inj.append((base + 46, qk_chunk(p, 0, 2)))
        inj.append((base + 50, qk_chunk(p, 0, 3)))
    inj.sort(key=lambda t: t[0])
    inj = deque(inj)

    # ---- the main software-pipelined stream ----
    st2s = {}
    cur_iter = [0]

    def normalize(sc, h, acc, acs):
        p, hl = h // 2, h % 2
        rec = recp.tile([128, SB8], F32, tag="rec", name=f"rec_{sc}_{h}")
        nc.vector.reciprocal(rec[:], acs[:])
        if hl == 0:
            st2s[(p, sc)] = st2p.tile([128, SB8, 128], F16, tag="st2",
                                      name=f"st2_{p}_{sc}")
        st2 = st2s[(p, sc)]
        for sb8 in range(SB8):
            nc.vector.tensor_scalar_mul(
                st2[:, sb8, hl * 64:(hl + 1) * 64],
                acc[:, sb8, :], rec[:, sb8:sb8 + 1])
        if hl == 1:
            for sb8 in range(SB8):
                nc.sync.dma_start_transpose(
                    concatT[:, p, sc * 1024 + sb8 * 128:
                            sc * 1024 + (sb8 + 1) * 128],
                    st2[:, sb8, :])
            if p == 0:
                oacc[sc] = outp.tile([128, SB8, 2, 512], F16, tag="oacc",
                                     name=f"oacc_{sc}")
            for sb8 in range(SB8):
                for ec in range(2):
                    injl.append((cur_iter[0], outproj_partial(sc, sb8, ec, p)))

    iters = [(sc, h, tb) for (sc, h) in UNITS for tb in range(TB)]
    pend = deque()
    accs = {}

    def flush_one():
        # psum accumulation "zero regions" are whole 2KB banks, so the 8
        # per-s-block groups in one bank can't use the start bit: zero the
        # banks once with the DVE, then accumulate with start=False.
        ex, sc, h, tb = pend.popleft()
        if tb == 0:
            acc = ps_acc.tile([128, SB8, DH], F32, tag="acc",
                              name=f"acc_{sc}_{h}")
            acs = ps_acc.tile([128, SB8], F32, tag="acs",
                              name=f"acs_{sc}_{h}")
            nc.vector.memset(acc[:], 0.0)
            nc.vector.memset(acs[:], 0.0)
            accs[(sc, h)] = (acc, acs)
        acc, acs = accs[(sc, h)]
        for sb8 in range(SB8):
            exb = ex[:, sb8 * 128:(sb8 + 1) * 128]
            nc.tensor.matmul(acc[:, sb8, :], exb, vext[:, tb, h, 0:DH],
                             start=False, stop=False, skip_group_check=True)
            nc.tensor.matmul(acs[:, sb8:sb8 + 1], exb,
                             vext[:, tb, h, DH:DH + 1],
                             start=False, stop=False, skip_group_check=True)
        if tb == TB - 1:
            normalize(sc, h, *accs.pop((sc, h)))

    def pump_inj(i):
        for n in range(3):
            src = None
            if inj and injl:
                src = inj if inj[0][0] <= injl[0][0] else injl
            elif inj or injl:
                src = inj or injl
            if src is None:
                return
            if n == 0 or src[0][0] <= i:
                src.popleft()[1]()
            else:
                return

    for i, (sc, h, tb) in enumerate(iters):
        cur_iter[0] = i
        hl, p = h % 2, h // 2
        scp = ps_slot.tile([128, 1024], F32, tag="slot", name=f"s_{i}")
        kblk = kT[hl * 64:(hl + 1) * 64, p, tb * 128:(tb + 1) * 128]
        for half in range(2):
            c0 = sc * 1024 + half * 512
            nc.tensor.matmul(scp[:, half * 512:(half + 1) * 512], kblk,
                             qT[hl * 64:(hl + 1) * 64, p, c0:c0 + 512],
                             start=True, stop=True)
        ex = exp_p.tile([128, 1024], F16, tag="ex", name=f"ex_{i}")
        nc.scalar.activation(ex[:], scp[:], Exp, scale=0.125)
        pend.append((ex, sc, h, tb))
        # unit 0 builds a backlog so vext projection gets a head start;
        # afterwards drain 2/iter until back to lag-1
        cap = 17 if i < 16 else max(2, 33 - i)
        nflush = 0
        while pend and len(pend) > cap - 1 and nflush < 2:
            flush_one()
            nflush += 1
        pump_inj(i)

    while pend:
        flush_one()
    while inj or injl:
        pump_inj(1 << 30)

    if dbg is not None:
        dbg_qt, dbg_kt, dbg_vx, dbg_cc, dbg_xt = dbg
        nc.sync.dma_start(dbg_qt[:], qT[:])
        nc.sync.dma_start(dbg_kt[:], kT[:])
        nc.sync.dma_start(dbg_vx[:], vext[:])
        nc.sync.dma_start(dbg_cc[:], concatT[:])
        nc.sync.dma_start(dbg_xt[:], xT[:])


_CACHE = {}


def _build(debug=False):
    nc = bacc.Bacc("TRN2", target_bir_lowering=False, debug=False,
                   num_devices=N_CORES)
    x_d = nc.dram_tensor("x", [S, E], F16, kind="ExternalInput").ap()
    wq_d = nc.dram_tensor("wq", [E, JW], F16, kind="ExternalInput").ap()
    wk_d = nc.dram_tensor("wk", [E, JW], F16, kind="ExternalInput").ap()
    wv_d = nc.dram_tensor("wv", [E, JW], F16, kind="ExternalInput").ap()
    wo_d = nc.dram_tensor("wo", [JW, E], F16, kind="ExternalInput").ap()
    bqt_d = nc.dram_tensor("bqt", [128, 4], F32, kind="ExternalInput").ap()
    bkt_d = nc.dram_tensor("bkt", [128, 4], F32, kind="ExternalInput").ap()
    bv_d = nc.dram_tensor("bv", [1, JW], F32, kind="ExternalInput").ap()
    out_d = nc.dram_tensor("out", [S, E], F16, kind="ExternalOutput").ap()
    aps = (x_d, wq_d, wk_d, wv_d, wo_d, bqt_d, bkt_d, bv_d, out_d)
    dbg = None
    if debug:
        dbg = (
            nc.dram_tensor("dbg_qt", [128, 4, S], F16,
                           kind="ExternalOutput").ap(),
            nc.dram_tensor("dbg_kt", [128, 4, S], F16,
                           kind="ExternalOutput").ap(),
            nc.dram_tensor("dbg_vx", [128, TB, HPC, DH + 1], F16,
                           kind="ExternalOutput").ap(),
            nc.dram_tensor("dbg_cc", [128, 4, S], F16,
                           kind="ExternalOutput").ap(),
            nc.dram_tensor("dbg_xt", [128, EB, S], F16,
                           kind="ExternalOutput").ap(),
        )
    with tile.TileContext(nc) as tc:
        with ExitStack() as ctx:
            _emit(tc, aps, ctx, dbg=dbg)
    nc.compile()
    return nc


def kernel(x, Wq, bq, Wk, bk, Wv, bv, Wo, bo):
    x = np.asarray(x, dtype=np.float32)
    Wq = np.asarray(Wq, dtype=np.float32)
    bq = np.asarray(bq, dtype=np.float32)
    Wk = np.asarray(Wk, dtype=np.float32)
    bk = np.asarray(bk, dtype=np.float32)
    Wv = np.asarray(Wv, dtype=np.float32)
    bv = np.asarray(bv, dtype=np.float32)
    Wo = np.asarray(Wo, dtype=np.float32)
    bo = np.asarray(bo, dtype=np.float32)

    if "nc" not in _CACHE:
        _CACHE["nc"] = _build()
    nc = _CACHE["nc"]

    WoT = np.ascontiguousarray(Wo.T)  # [f, e]
    in_maps = []
    for c in range(N_CORES):
        b, hh = c // 2, c % 2
        hs = slice(hh * HPC, (hh + 1) * HPC)
        in_maps.append({
            "x": np.ascontiguousarray(x[b]).astype(np.float16),
            "wq": np.ascontiguousarray(
                Wq[hs].transpose(1, 0, 2).reshape(E, JW)).astype(np.float16),
            "wk": np.ascontiguousarray(
                Wk[hs].transpose(1, 0, 2).reshape(E, JW)).astype(np.float16),
            "wv": np.ascontiguousarray(
                Wv[hs].transpose(1, 0, 2).reshape(E, JW)).astype(np.float16),
            "wo": np.ascontiguousarray(
                WoT[hh * JW:(hh + 1) * JW]).astype(np.float16),
            "bqt": np.ascontiguousarray(bq[hs].reshape(4, 128).T),
            "bkt": np.ascontiguousarray(bk[hs].reshape(4, 128).T),
            "bv": bv[hs].reshape(1, JW),
        })

    res = bass_utils.run_bass_kernel_spmd(nc, in_maps,
                                          core_ids=list(range(N_CORES)))
    out = np.empty((B, S, E), dtype=np.float32)
    for b in range(B):
        out[b] = (res.results[2 * b]["out"].astype(np.float32)
                  + res.results[2 * b + 1]["out"].astype(np.float32))
        out[b] += bo[None, :]
    return out


# revision 31
# speedup vs baseline: 1.1113x; 1.1113x over previous
"""Multi-head attention kernel for Trainium2, 8 NeuronCores.

Problem (hardcoded shapes): B=4, S=2048, E=1024, H=16, DH=64.
  q/k/v = einsum('bse,hed->bhsd', x, W{q,k,v}) + b{q,k,v}
  attn  = softmax(q k^T / sqrt(DH)) v
  out   = concat_heads(attn) @ Wo^T + bo

Sharding: core c -> (batch b = c//2, head-half hh = c%2, i.e. heads
8*hh..8*hh+7).  Each core computes a [S, E] partial of its batch's output;
the host sums the two partials per batch and adds bo.

Everything 2-byte (fp16) on the PE so every matmul runs at 1 col/cycle and
the DMA xbar can do the transposes:
  xT   [e=128 x 8, s=2048]  f16, via dma_start_transpose (no PE/DVE cost)
  qT/kT [j=128, pair, s]    f16, Wq^T x + bias per pair (PE + DVE bias)
  vext [t, tb, h, 65]       f16, x Wv + bias, with a fused ones column
  scores [t=128, s=1024]    PSUM f32, one matmul per (h, s-chunk, t-block)
  exp   [t=128, s=1024]     ACT Exp(scale=1/8) -> SBUF f16
  attnV FLIPPED: out [s=128, 65] += exp-block^T (lhsT) @ vext[t,65] (rhs);
        65-col moving operand -> half the PE cost of the [65, s] orientation
  normalize: DVE recip on the sums column + per-partition tensor_scalar
  concatT via dma_start_transpose of the normalized [s, f] blocks
  outproj: per-pair partial matmuls accumulated in SBUF f32 by DVE, so the
        last pair's work is the only tail
"""

import os
import sys

for _p in ("/opt/trn_rl_repo", "/root/.axon_site/_ro/trn_rl_repo"):
    if os.path.isdir(_p) and _p not in sys.path:
        sys.path.insert(0, _p)
        break

from collections import deque
from contextlib import ExitStack

import numpy as np

import concourse.bass as bass
import concourse.tile as tile
import concourse.mybir as mybir
from concourse import bacc, bass_utils

B, S, E, H, DH = 4, 2048, 1024, 16, 64
HPC = 8           # heads per core
JW = HPC * DH     # 512, per-core qkv width
N_CORES = 8
EB = E // 128     # 8 e-blocks
TB = S // 128     # 16 t-blocks
SB8 = 8           # s-blocks per 1024-wide s-chunk
F32 = mybir.dt.float32
F16 = mybir.dt.float16
Exp = mybir.ActivationFunctionType.Exp
ADD = mybir.AluOpType.add

# (sc, h) processing order: interleave the two s-chunks so each qk pair's
# projection window is ~4 units wide and outproj partials spread out.
UNITS = []
for hp in range(4):
    UNITS += [(0, 2 * hp), (0, 2 * hp + 1), (1, 2 * hp), (1, 2 * hp + 1)]


def _emit(tc, aps, ctx, dbg=None):
    nc = tc.nc
    x_d, wq_d, wk_d, wv_d, wo_d, bqt_d, bkt_d, bv_d, out_d = aps

    def pool(**kw):
        return ctx.enter_context(tc.tile_pool(**kw))

    const = pool(name="const", bufs=1)
    xTp = pool(name="xT", bufs=1)
    vxp = pool(name="vext", bufs=1)
    wqk = pool(name="wqk", bufs=2)
    qkp = pool(name="qk", bufs=1)
    exp_p = pool(name="expS", bufs=20)
    st2p = pool(name="st2", bufs=2)
    recp = pool(name="rec", bufs=2)
    ccp = pool(name="concatT", bufs=1)
    outp = pool(name="outs", bufs=2)
    ps_slot = pool(name="ps_slot", bufs=2, space="PSUM")
    ps_acc = pool(name="ps_acc", bufs=1, space="PSUM")
    ps_proj = pool(name="ps_proj", bufs=1, space="PSUM")

    # ---- pair-0 weights first on SP, then the x transposes split across the
    # two HWDGE engines so they all issue within ~3us ----
    wq_r = wq_d.rearrange("(eb pp) j -> pp eb j", pp=128)
    wk_r = wk_d.rearrange("(eb pp) j -> pp eb j", pp=128)

    # NOTE: the DMA-transpose xbar is a single shared unit — concurrent
    # transposes issued from different HWDGE queues corrupt each other
    # (verified empirically).  ALL DMAs go through nc.sync, in the order the
    # pipeline consumes them: tiny biases + pair-0 weights, then the eight x
    # transposes (the first score needs all of them), then wv/wo.
    def load_pair_weights(p):
        wq_t = wqk.tile([128, EB, 128], F16, tag="wq", name=f"wq{p}")
        nc.sync.dma_start(wq_t[:], wq_r[:, :, p * 128:(p + 1) * 128])
        wk_t = wqk.tile([128, EB, 128], F16, tag="wk", name=f"wk{p}")
        nc.sync.dma_start(wk_t[:], wk_r[:, :, p * 128:(p + 1) * 128])
        return wq_t, wk_t

    bq_sb = const.tile([128, 4], F32)
    nc.sync.dma_start(bq_sb[:], bqt_d[:])
    bk_sb = const.tile([128, 4], F32)
    nc.sync.dma_start(bk_sb[:], bkt_d[:])
    bv1 = const.tile([1, JW], F32)
    nc.sync.dma_start(bv1[:], bv_d[:])

    pair_w = {0: load_pair_weights(0)}

    xT = xTp.tile([128, EB, S], F16)
    for eb in range(EB):
        nc.sync.dma_start_transpose(xT[:, eb, :], x_d[:, eb * 128:(eb + 1) * 128])

    wv_sb = const.tile([128, EB, JW], F16)
    nc.sync.dma_start(wv_sb[:], wv_d.rearrange("(eb p) j -> p eb j", p=128))
    wo_sb = const.tile([128, 4, E], F16)
    nc.sync.dma_start(wo_sb[:], wo_d.rearrange("(fb p) e -> p fb e", p=128))
    bvb = const.tile([128, JW], F32)
    nc.gpsimd.partition_broadcast(bvb[:], bv1[:])

    vext = vxp.tile([128, TB, HPC, DH + 1], F16)
    nc.gpsimd.memset(vext[:, :, :, DH:DH + 1], 1.0)

    # qT/kT for all 4 pairs stay resident (f16, 16KB/partition total)
    qT = qkp.tile([128, 4, S], F16, tag="qT")
    kT = qkp.tile([128, 4, S], F16, tag="kT")

    # ---- injectable PE work chunks ----
    def qk_chunk(p, qk, c):
        """One 512-col q/k projection chunk (8 e-block matmuls + bias)."""
        def emit():
            wq_t, wk_t = pair_w[p]
            w_t, dst, b_sb = ((wq_t, qT, bq_sb) if qk == 0 else
                              (wk_t, kT, bk_sb))
            pq = ps_proj.tile([128, 512], F32, tag="pq",
                              name=f"pq_{p}_{qk}_{c}")
            for eb in range(EB):
                nc.tensor.matmul(pq[:], w_t[:, eb, :],
                                 xT[:, eb, c * 512:(c + 1) * 512],
                                 start=(eb == 0), stop=(eb == EB - 1))
            nc.vector.tensor_scalar_add(
                dst[:, p, c * 512:(c + 1) * 512], pq[:], b_sb[:, p:p + 1])
        return emit

    def v_chunk(tb, hp):
        """Project v for t-block tb, head pair hp (2 heads, 128 cols)."""
        def emit():
            pv = ps_proj.tile([128, 128], F32, tag="proj",
                              name=f"pv_{tb}_{hp}")
            for eb in range(EB):
                nc.tensor.matmul(pv[:], xT[:, eb, tb * 128:(tb + 1) * 128],
                                 wv_sb[:, eb, hp * 128:(hp + 1) * 128],
                                 start=(eb == 0), stop=(eb == EB - 1))
            nc.vector.tensor_tensor(
                vext[:, tb, 2 * hp:2 * hp + 2, 0:DH],
                pv[:].rearrange("p (h d) -> p h d", h=2),
                bvb[:, hp * 128:(hp + 1) * 128].rearrange(
                    "p (h d) -> p h d", h=2), ADD)
        return emit

    concatT = ccp.tile([128, 4, S], F16)
    oacc = {}  # sc -> SBUF f32 accumulator

    def outproj_partial(sc, sb8, ec, p):
        def emit():
            po = ps_proj.tile([128, 512], F32, tag="proj",
                              name=f"po_{sc}_{sb8}_{ec}_{p}")
            nc.tensor.matmul(po[:],
                             concatT[:, p, sc * 1024 + sb8 * 128:
                                     sc * 1024 + (sb8 + 1) * 128],
                             wo_sb[:, p, ec * 512:(ec + 1) * 512],
                             start=True, stop=True)
            oa = oacc[sc][:, sb8, ec, :]
            if p == 0:
                nc.vector.tensor_copy(oa, po[:])
            else:
                nc.vector.tensor_tensor(oa, po[:], oa, ADD)
            if p == 3:
                r0 = sc * 1024 + sb8 * 128
                nc.sync.dma_start(out_d[r0:r0 + 128, ec * 512:(ec + 1) * 512],
                                  oacc[sc][:, sb8, ec, :])
        return emit

    # k chunk 0 / q chunks 0,1 of pair 0 and v t-block 0 are needed by the
    # very first attention iterations: emit them in the prefix.
    qk_chunk(0, 1, 0)()
    qk_chunk(0, 0, 0)()
    qk_chunk(0, 0, 1)()
    v_chunk(0, 0)()

    # ---- build the due-scheduled injection queue ----
    inj = []   # (due_iter, fn) pre-sorted
    injl = deque()  # runtime-enqueued (outproj partials), FIFO

    # pair0 remaining chunks (k s4-1..3 urgent for t-blocks 4+, q s4-2,3 by
    # iter 32 when s-chunk 1 starts)
    for c, due in ((1, 2), (2, 6), (3, 10)):
        inj.append((due, qk_chunk(0, 1, c)))
    for c, due in ((2, 16), (3, 20)):
        inj.append((due, qk_chunk(0, 0, c)))
    # v chunks: heads 2hp..2hp+1 first consumed from unit 4*hp (iter 64*hp);
    # unit 0's attnV is deferred ~16 iters so h0/h1 dues can stay sparse
    for tb in range(1, TB):
        inj.append((tb + 2 if tb < 8 else 4 + tb, v_chunk(tb, 0)))
    for hp, base in ((1, 40), (2, 104), (3, 168)):
        for tb in range(TB):
            inj.append((base + tb, v_chunk(tb, hp)))
    # pairs 1..3: weights load + 8 projection chunks each
    for p, base in ((1, 26), (2, 90), (3, 154)):
        def mk_load(p=p):
            def emit():
                pair_w[p] = load_pair_weights(p)
            return emit
        inj.append((base - 2, mk_load()))
        seq = [(1, 0), (0, 0), (0, 1), (1, 1), (1, 2), (1, 3)]
        for i, (qk, c) in enumerate(seq):
            inj.append((base + 4 * i, qk_chunk(p, qk, c)))
        inj.append((base + 28, qk_chunk(p, 0, 2)))
        inj.append((base + 32, qk_chunk(p, 0, 3)))
    inj.sort(key=lambda t: t[0])
    inj = deque(inj)

    # ---- the main software-pipelined stream ----
    st2s = {}
    cur_iter = [0]

    def normalize(sc, h, acc):
        p, hl = h // 2, h % 2
        rec = recp.tile([128, 2, 4], F32, tag="rec", name=f"rec_{sc}_{h}")
        nc.vector.reciprocal(rec[:, 0, :], acc[0][:, :, DH:DH + 1])
        nc.vector.reciprocal(rec[:, 1, :], acc[1][:, :, DH:DH + 1])
        if hl == 0:
            st2s[(p, sc)] = st2p.tile([128, SB8, 128], F16, tag="st2",
                                      name=f"st2_{p}_{sc}")
        st2 = st2s[(p, sc)]
        for sb8 in range(SB8):
            nc.vector.tensor_scalar_mul(
                st2[:, sb8, hl * 64:(hl + 1) * 64],
                acc[sb8 // 4][:, sb8 % 4, 0:DH],
                rec[:, sb8 // 4, sb8 % 4:sb8 % 4 + 1])
        if hl == 1:
            for sb8 in range(SB8):
                nc.sync.dma_start_transpose(
                    concatT[:, p, sc * 1024 + sb8 * 128:
                            sc * 1024 + (sb8 + 1) * 128],
                    st2[:, sb8, :])
            if p == 0:
                oacc[sc] = outp.tile([128, SB8, 2, 512], F16, tag="oacc",
                                     name=f"oacc_{sc}")
            for sb8 in range(SB8):
                for ec in range(2):
                    injl.append((cur_iter[0], outproj_partial(sc, sb8, ec, p)))

    iters = [(sc, h, tb) for (sc, h) in UNITS for tb in range(TB)]
    pend = deque()
    accs = {}

    def flush_one():
        # psum accumulation "zero regions" are whole 2KB banks, so the 4
        # per-s-block groups in one bank can't use the start bit: zero the
        # banks once with the DVE, then accumulate with start=False.
        ex, sc, h, tb = pend.popleft()
        if tb == 0:
            acc = (ps_acc.tile([128, 4, DH + 1], F32, tag="acca",
                               name=f"acca_{sc}_{h}"),
                   ps_acc.tile([128, 4, DH + 1], F32, tag="accb",
                               name=f"accb_{sc}_{h}"))
            nc.vector.memset(acc[0][:], 0.0)
            nc.vector.memset(acc[1][:], 0.0)
            accs[(sc, h)] = acc
        acc = accs[(sc, h)]
        for sb8 in range(SB8):
            nc.tensor.matmul(acc[sb8 // 4][:, sb8 % 4, :],
                             ex[:, sb8 * 128:(sb8 + 1) * 128],
                             vext[:, tb, h, :],
                             start=False, stop=False, skip_group_check=True)
        if tb == TB - 1:
            normalize(sc, h, accs.pop((sc, h)))

    def pump_inj(i, budget=1):
        # at most one chunk per iteration: a burst of PE work between two
        # score matmuls delays the pong-slot refill and stalls the ACT
        for _ in range(budget):
            src = None
            if inj and injl:
                src = inj if inj[0][0] <= injl[0][0] else injl
            elif inj or injl:
                src = inj or injl
            if src is None:
                return
            src.popleft()[1]()

    for i, (sc, h, tb) in enumerate(iters):
        cur_iter[0] = i
        hl, p = h % 2, h // 2
        scp = ps_slot.tile([128, 1024], F32, tag="slot", name=f"s_{i}")
        kblk = kT[hl * 64:(hl + 1) * 64, p, tb * 128:(tb + 1) * 128]
        for half in range(2):
            c0 = sc * 1024 + half * 512
            nc.tensor.matmul(scp[:, half * 512:(half + 1) * 512], kblk,
                             qT[hl * 64:(hl + 1) * 64, p, c0:c0 + 512],
                             start=True, stop=True)
        ex = exp_p.tile([128, 1024], F16, tag="ex", name=f"ex_{i}")
        nc.scalar.activation(ex[:], scp[:], Exp, scale=0.125)
        pend.append((ex, sc, h, tb))
        # unit 0 builds a backlog so vext projection gets a head start;
        # afterwards drain 2/iter until back to lag-1
        cap = 17 if i < 16 else max(2, 33 - i)
        nflush = 0
        while pend and len(pend) > cap - 1 and nflush < 2:
            flush_one()
            nflush += 1
        pump_inj(i)

    while pend:
        flush_one()
    while inj or injl:
        pump_inj(1 << 30, budget=4)

    if dbg is not None:
        dbg_qt, dbg_kt, dbg_vx, dbg_cc, dbg_xt = dbg
        nc.sync.dma_start(dbg_qt[:], qT[:])
        nc.sync.dma_start(dbg_kt[:], kT[:])
        nc.sync.dma_start(dbg_vx[:], vext[:])
        nc.sync.dma_start(dbg_cc[:], concatT[:])
        nc.sync.dma_start(dbg_xt[:], xT[:])


_CACHE = {}


def _build(debug=False):
    nc = bacc.Bacc("TRN2", target_bir_lowering=False, debug=False,
                   num_devices=N_CORES)
    x_d = nc.dram_tensor("x", [S, E], F16, kind="ExternalInput").ap()
    wq_d = nc.dram_tensor("wq", [E, JW], F16, kind="ExternalInput").ap()
    wk_d = nc.dram_tensor("wk", [E, JW], F16, kind="ExternalInput").ap()
    wv_d = nc.dram_tensor("wv", [E, JW], F16, kind="ExternalInput").ap()
    wo_d = nc.dram_tensor("wo", [JW, E], F16, kind="ExternalInput").ap()
    bqt_d = nc.dram_tensor("bqt", [128, 4], F32, kind="ExternalInput").ap()
    bkt_d = nc.dram_tensor("bkt", [128, 4], F32, kind="ExternalInput").ap()
    bv_d = nc.dram_tensor("bv", [1, JW], F32, kind="ExternalInput").ap()
    out_d = nc.dram_tensor("out", [S, E], F16, kind="ExternalOutput").ap()
    aps = (x_d, wq_d, wk_d, wv_d, wo_d, bqt_d, bkt_d, bv_d, out_d)
    dbg = None
    if debug:
        dbg = (
            nc.dram_tensor("dbg_qt", [128, 4, S], F16,
                           kind="ExternalOutput").ap(),
            nc.dram_tensor("dbg_kt", [128, 4, S], F16,
                           kind="ExternalOutput").ap(),
            nc.dram_tensor("dbg_vx", [128, TB, HPC, DH + 1], F16,
                           kind="ExternalOutput").ap(),
            nc.dram_tensor("dbg_cc", [128, 4, S], F16,
                           kind="ExternalOutput").ap(),
            nc.dram_tensor("dbg_xt", [128, EB, S], F16,
                           kind="ExternalOutput").ap(),
        )
    with tile.TileContext(nc) as tc:
        with ExitStack() as ctx:
            _emit(tc, aps, ctx, dbg=dbg)
    nc.compile()
    return nc


def kernel(x, Wq, bq, Wk, bk, Wv, bv, Wo, bo):
    x = np.asarray(x, dtype=np.float32)
    Wq = np.asarray(Wq, dtype=np.float32)
    bq = np.asarray(bq, dtype=np.float32)
    Wk = np.asarray(Wk, dtype=np.float32)
    bk = np.asarray(bk, dtype=np.float32)
    Wv = np.asarray(Wv, dtype=np.float32)
    bv = np.asarray(bv, dtype=np.float32)
    Wo = np.asarray(Wo, dtype=np.float32)
    bo = np.asarray(bo, dtype=np.float32)

    if "nc" not in _CACHE:
        _CACHE["nc"] = _build()
    nc = _CACHE["nc"]

    WoT = np.ascontiguousarray(Wo.T)  # [f, e]
    in_maps = []
    for c in range(N_CORES):
        b, hh = c // 2, c % 2
        hs = slice(hh * HPC, (hh + 1) * HPC)
        in_maps.append({
            "x": np.ascontiguousarray(x[b]).astype(np.float16),
            "wq": np.ascontiguousarray(
                Wq[hs].transpose(1, 0, 2).reshape(E, JW)).astype(np.float16),
            "wk": np.ascontiguousarray(
                Wk[hs].transpose(1, 0, 2).reshape(E, JW)).astype(np.float16),
            "wv": np.ascontiguousarray(
                Wv[hs].transpose(1, 0, 2).reshape(E, JW)).astype(np.float16),
            "wo": np.ascontiguousarray(
                WoT[hh * JW:(hh + 1) * JW]).astype(np.float16),
            "bqt": np.ascontiguousarray(bq[hs].reshape(4, 128).T),
            "bkt": np.ascontiguousarray(bk[hs].reshape(4, 128).T),
            "bv": bv[hs].reshape(1, JW),
        })

    res = bass_utils.run_bass_kernel_spmd(nc, in_maps,
                                          core_ids=list(range(N_CORES)))
    out = np.empty((B, S, E), dtype=np.float32)
    for b in range(B):
        out[b] = (res.results[2 * b]["out"].astype(np.float32)
                  + res.results[2 * b + 1]["out"].astype(np.float32))
        out[b] += bo[None, :]
    return out


# revision 63
# speedup vs baseline: 1.2074x; 1.0864x over previous
"""Multi-head attention kernel for Trainium2, 8 NeuronCores.

Problem (hardcoded shapes): B=4, S=2048, E=1024, H=16, DH=64.
  q/k/v = einsum('bse,hed->bhsd', x, W{q,k,v}) + b{q,k,v}
  attn  = softmax(q k^T / sqrt(DH)) v
  out   = concat_heads(attn) @ Wo^T + bo

Sharding: core c -> (batch b = c//2, head-half hh = c%2, i.e. heads
8*hh..8*hh+7).  Each core computes a [S, E] partial of its batch's output;
the host sums the two partials per batch and adds bo.

Everything 2-byte (fp16) on the PE so every matmul runs at 1 col/cycle:
  xT   [e=128 x 8, s=2048]  f16, PE-transpose of DMA'd x s-blocks
  qT/kT [j=128, pair, s]    f16, Wq^T x + bias per pair (PE + DVE bias)
  vext [t, tb, h, 65]       f16, x Wv + bias, with a fused ones column
  scores [t=128, s=1024]    PSUM f32, two 512-col matmuls per group
  exp   [t=128, s=1024]     one ACT Exp(scale=1/8) instr -> SBUF f16
  attnV FLIPPED: out[s=128, 65] += exp-block (lhsT) @ vext[t,65] (rhs);
        65-col moving operand -> half the PE cost of the [65, s] orientation.
        PSUM accumulation uses DVE-zeroed banks + start=False because the
        hardware "zero region" is a whole bank (4 groups share each bank).
  normalize: DVE recip on the sums column + per-s-block tensor_scalar
  concatT via PE-transpose of the normalized [s, f] blocks
  outproj: per-pair partial matmuls accumulated in SBUF f16 by the DVE, so
        only the last pair's 16 partials sit in the tail

Scheduling: one global software-pipelined stream over (s-chunk, head,
t-block) groups; projection/outproj work is broken into ~0.4-0.9us chunks
and injected one per iteration from a due-time queue.  Every consumer
*pulls* (emit-once) its producer chunks first, so emission order is correct
by construction regardless of the due tuning.

NOTE: dma_start_transpose is avoided entirely — its completion semaphore
fires before the data lands (verified empirically: a matmul chasing the
transpose reads garbage), and concurrent xbar transposes corrupt.
"""

import os
import sys

for _p in ("/opt/trn_rl_repo", "/root/.axon_site/_ro/trn_rl_repo"):
    if os.path.isdir(_p) and _p not in sys.path:
        sys.path.insert(0, _p)
        break

from collections import deque
from contextlib import ExitStack

import numpy as np

import concourse.bass as bass
import concourse.tile as tile
import concourse.mybir as mybir
from concourse import bacc, bass_utils

B, S, E, H, DH = 4, 2048, 1024, 16, 64
HPC = 8           # heads per core
JW = HPC * DH     # 512, per-core qkv width
N_CORES = 8
EB = E // 128     # 8 e-blocks
TB = S // 128     # 16 t-blocks (also x s-blocks)
SB8 = 8           # s-blocks per 1024-wide s-chunk
F32 = mybir.dt.float32
F16 = mybir.dt.float16
Exp = mybir.ActivationFunctionType.Exp
ADD = mybir.AluOpType.add

# (sc, h) processing order: interleave the two s-chunks so each qk pair's
# projection window is ~4 units wide and outproj partials spread out.
UNITS = []
for hp in range(4):
    UNITS += [(0, 2 * hp), (0, 2 * hp + 1), (1, 2 * hp), (1, 2 * hp + 1)]


def _emit(tc, aps, ctx, dbg=None):
    nc = tc.nc
    x_d, wq_d, wk_d, wv_d, wo_d, bqt_d, bkt_d, bv_d, id_d, out_d = aps

    def pool(**kw):
        return ctx.enter_context(tc.tile_pool(**kw))

    const = pool(name="const", bufs=1)
    xs = pool(name="xs", bufs=1)
    xTp = pool(name="xT", bufs=1)
    vxp = pool(name="vext", bufs=1)
    wqk = pool(name="wqk", bufs=2)
    qkp = pool(name="qk", bufs=1)
    exp_p = pool(name="expS", bufs=17)
    st2p = pool(name="st2", bufs=2)
    recp = pool(name="rec", bufs=2)
    ccp = pool(name="concatT", bufs=1)
    outp = pool(name="outs", bufs=2)
    ps_slot = pool(name="ps_slot", bufs=2, space="PSUM")
    ps_acc = pool(name="ps_acc", bufs=1, space="PSUM")
    ps_proj = pool(name="ps_proj", bufs=1, space="PSUM")

    # ---- DMAs, in consumption order, all on the sync queue ----
    wq_r = wq_d.rearrange("(eb pp) j -> pp eb j", pp=128)
    wk_r = wk_d.rearrange("(eb pp) j -> pp eb j", pp=128)

    def load_pair_weights(p):
        wk_t = wqk.tile([128, EB, 128], F16, tag="wk", name=f"wk{p}")
        nc.sync.dma_start(wk_t[:], wk_r[:, :, p * 128:(p + 1) * 128])
        wq_t = wqk.tile([128, EB, 128], F16, tag="wq", name=f"wq{p}")
        nc.sync.dma_start(wq_t[:], wq_r[:, :, p * 128:(p + 1) * 128])
        return wq_t, wk_t

    wv_sb = const.tile([128, EB, JW], F16)
    wo_sb = const.tile([128, 4, E], F16)

    # x staged in four 4-s-block group DMAs (one instruction each), with the
    # weight loads slotted between them in consumption order
    x_groups = {}
    x_r = x_d.rearrange("(g p) e -> p g e", p=128)

    def dma_xg(g):
        x_t = xs.tile([128, 4, E], F16, tag="x_t", name=f"x_g_{g}")
        nc.sync.dma_start(x_t[:], x_r[:, 4 * g:4 * g + 4, :])
        x_groups[g] = x_t

    dma_xg(0)
    ident = const.tile([128, 128], F16)
    nc.sync.dma_start(ident[:], id_d[:])
    bq_sb = const.tile([128, 4], F32)
    nc.sync.dma_start(bq_sb[:], bqt_d[:])
    bk_sb = const.tile([128, 4], F32)
    nc.sync.dma_start(bk_sb[:], bkt_d[:])
    bv1 = const.tile([1, JW], F32)
    nc.sync.dma_start(bv1[:], bv_d[:])
    pair_w = {0: load_pair_weights(0)}
    dma_xg(1)
    dma_xg(2)
    dma_xg(3)
    nc.sync.dma_start(wv_sb[:], wv_d.rearrange("(eb p) j -> p eb j", p=128))
    nc.sync.dma_start(wo_sb[:], wo_d.rearrange("(fb p) e -> p fb e", p=128))

    # PE p-state warmup: the tensor engine ramps to full clock only after
    # ~3us of sustained use, and the ramp clock starts at the first busy
    # period.  Run throwaway matmuls from t~0.3us so the real prefix work
    # (from ~4.5us, when x group 0 lands) runs at full speed.
    warm = const.tile([128, 512], F16)
    nc.vector.memset(warm[:], 0.25)
    for i in range(18):
        pw = ps_slot.tile([128, 512], F32, tag="slot", name=f"warm{i}")
        nc.tensor.matmul(pw[:], warm[:, 0:128], warm[:], start=True, stop=True)

    bvb = const.tile([128, JW], F32)
    nc.gpsimd.partition_broadcast(bvb[:], bv1[:])

    vext = vxp.tile([128, TB, HPC, DH + 1], F16)
    nc.gpsimd.memset(vext[:, :, :, DH:DH + 1], 1.0)

    qT = qkp.tile([128, 4, S], F16, tag="qT")
    kT = qkp.tile([128, 4, S], F16, tag="kT")
    xT = xTp.tile([128, EB, S], F16)
    concatT = ccp.tile([128, 4, S], F16)

    # ---- emit-once chunk machinery: consumers pull producers ----
    emitted = set()

    def once(key, deps, fn):
        def run():
            if key in emitted:
                return
            emitted.add(key)
            for d in deps():
                d()
            fn()
        return run

    def x_chunk(sb):
        """PE-transpose x s-block sb into xT.  Uses the score-slot psum pool
        (2 buffers, idle during the prefix) so consecutive chunks double-
        buffer instead of serializing on a single-bank WAR chain."""
        def emit():
            x_t = x_groups[sb // 4][:, sb % 4, :]
            pt = ps_slot.tile([128, 1024], F16, tag="slot", name=f"pt_{sb}")
            for eb in range(EB):
                nc.tensor.transpose(pt[:, eb * 128:(eb + 1) * 128],
                                    x_t[:, eb * 128:(eb + 1) * 128],
                                    ident[:])
            nc.vector.tensor_copy(
                xT[:, :, sb * 128:(sb + 1) * 128],
                pt[:].rearrange("p (e s) -> p e s", e=8))
        return once(("x", sb), lambda: [], emit)

    _qk_ps = {}
    _qk_open = [None]

    def qk_half(p, qk, c, half):
        """Half (4 e-blocks) of one 512-col q/k projection chunk.  The "pq"
        psum pool has one buffer, so before opening a new chunk any other
        half-open chunk is closed first."""
        def emit():
            if half == 0 and _qk_open[0] is not None:
                op, oqk, oc = _qk_open[0]
                qk_half(op, oqk, oc, 1)()
            wq_t, wk_t = pair_w[p]
            w_t, dst, b_sb = ((wq_t, qT, bq_sb) if qk == 0 else
                              (wk_t, kT, bk_sb))
            if half == 0:
                pq = ps_proj.tile([128, 512], F32, tag="pq",
                                  name=f"pq_{p}_{qk}_{c}")
                _qk_ps[(p, qk, c)] = pq
                _qk_open[0] = (p, qk, c)
            else:
                pq = _qk_ps.pop((p, qk, c))
                _qk_open[0] = None
            for q in range(4):
                eb = half * 4 + q
                nc.tensor.matmul(pq[:], w_t[:, eb, :],
                                 xT[:, eb, c * 512:(c + 1) * 512],
                                 start=(eb == 0), stop=(eb == EB - 1))
            if half == 1:
                nc.vector.tensor_scalar_add(
                    dst[:, p, c * 512:(c + 1) * 512], pq[:], b_sb[:, p:p + 1])

        def deps():
            d = [x_chunk(sb) for sb in range(4 * c, 4 * c + 4)]
            if half == 1:
                d.append(qk_half(p, qk, c, 0))
            return d
        return once(("qk", p, qk, c, half), deps, emit)

    def v_chunk(tb, hp):
        """Project v for t-block tb, head pair hp (2 heads, 128 cols).
        Alternates between the proj and (when no q/k chunk is half-open)
        pq psum banks so consecutive chunks double-buffer."""
        def emit():
            pv = ps_proj.tile([128, 128], F32, tag="proj",
                              name=f"pv_{tb}_{hp}")
            for eb in range(EB):
                nc.tensor.matmul(pv[:], xT[:, eb, tb * 128:(tb + 1) * 128],
                                 wv_sb[:, eb, hp * 128:(hp + 1) * 128],
                                 start=(eb == 0), stop=(eb == EB - 1))
            nc.vector.tensor_tensor(
                vext[:, tb, 2 * hp:2 * hp + 2, 0:DH],
                pv[:].rearrange("p (h d) -> p h d", h=2),
                bvb[:, hp * 128:(hp + 1) * 128].rearrange(
                    "p (h d) -> p h d", h=2), ADD)
        return once(("v", tb, hp), lambda: [x_chunk(tb)], emit)

    oacc = {}

    def outproj_partial(sc, sb8, ec, p):
        def emit():
            # the last pair of the last s-chunk is the kernel tail: ping-pong
            # through the freed "pq" bank, and route half the accumulates
            # through the idle ACT engine (PE identity-matmul adds oacc into
            # the psum group, ACT copies it out) so DVE and ACT split the work
            tail = (p == 3 and sc == 1)
            tag = "pq" if (tail and (sb8 + ec) % 2) else "proj"
            po = ps_proj.tile([128, 512], F32, tag=tag,
                              name=f"po_{sc}_{sb8}_{ec}_{p}")
            oa = oacc[sc][:, sb8, ec, :]
            act_lane = tail and (sb8 + ec) % 2
            nc.tensor.matmul(po[:],
                             concatT[:, p, sc * 1024 + sb8 * 128:
                                     sc * 1024 + (sb8 + 1) * 128],
                             wo_sb[:, p, ec * 512:(ec + 1) * 512],
                             start=True, stop=not act_lane)
            if act_lane:
                nc.tensor.matmul(po[:], ident[:], oa, start=False, stop=True)
                nc.scalar.copy(oa, po[:])
            elif p == 0:
                nc.vector.tensor_copy(oa, po[:])
            else:
                nc.vector.tensor_tensor(oa, po[:], oa, ADD)
            if p == 3 and ec == 1:
                r0 = sc * 1024 + sb8 * 128
                nc.sync.dma_start(out_d[r0:r0 + 128, :],
                                  oacc[sc][:, sb8, :, :])
        return once(("op", sc, sb8, ec, p), lambda: [], emit)

    # ---- prefix PE work: x transposes chase the x DMAs; pair-0 k/q chunks
    # chase the transposes e-block by e-block ----
    for sb in range(4):
        x_chunk(sb)()
    qk_half(0, 1, 0, 0)(); qk_half(0, 1, 0, 1)()
    qk_half(0, 0, 0, 0)(); qk_half(0, 0, 0, 1)()
    for sb in range(4, 8):
        x_chunk(sb)()
    qk_half(0, 0, 1, 0)(); qk_half(0, 0, 1, 1)()

    # ---- due-scheduled injection queue (performance tuning only) ----
    inj = []
    injl = deque()

    def qk_halves(p, qk, c, due):
        inj.append((due, qk_half(p, qk, c, 0)))
        inj.append((due, qk_half(p, qk, c, 1)))

    for sb in range(8, TB):
        inj.append((sb - 4, x_chunk(sb)))
    qk_halves(0, 1, 1, 2)
    qk_halves(0, 1, 2, 8)
    qk_halves(0, 1, 3, 12)
    qk_halves(0, 0, 2, 17)
    qk_halves(0, 0, 3, 21)
    # unit 0's attnV is deferred ~16 iters, so vext t-blocks are only needed
    # from iter ~16+tb
    for tb in range(1, TB):
        inj.append((11 + tb, v_chunk(tb, 0)))
    for hp, base in ((1, 38), (2, 98), (3, 162)):
        for tb in range(TB):
            inj.append((base + tb, v_chunk(tb, hp)))
    for p, base in ((1, 34), (2, 94), (3, 158)):
        def mk_load(p=p):
            def emit():
                pair_w[p] = load_pair_weights(p)
            return once(("wld", p), lambda: [], emit)
        inj.append((base - 2, mk_load()))
        seq = [(1, 0), (0, 0), (0, 1), (1, 1), (1, 2), (1, 3)]
        for i, (qk, c) in enumerate(seq):
            qk_halves(p, qk, c, base + 3 * i)
        qk_halves(p, 0, 2, base + 24)
        qk_halves(p, 0, 3, base + 28)
    inj.sort(key=lambda t: t[0])
    inj = deque(inj)

    def pump_inj(i, budget=1):
        for _ in range(budget):
            src = None
            if inj and injl:
                src = inj if inj[0][0] <= injl[0][0] else injl
            elif inj or injl:
                src = inj or injl
            if src is None:
                return
            src.popleft()[1]()

    # ---- the main software-pipelined stream ----
    st2s = {}
    cur_iter = [0]

    def normalize(sc, h, acc):
        p, hl = h // 2, h % 2
        rec = recp.tile([128, 2, 4], F32, tag="rec", name=f"rec_{sc}_{h}")
        nc.vector.reciprocal(rec[:, 0, :], acc[0][:, :, DH:DH + 1])
        nc.vector.reciprocal(rec[:, 1, :], acc[1][:, :, DH:DH + 1])
        if hl == 0:
            st2s[(p, sc)] = st2p.tile([128, SB8, 128], F16, tag="st2",
                                      name=f"st2_{p}_{sc}")
        st2 = st2s[(p, sc)]
        for sb8 in range(SB8):
            nc.vector.tensor_scalar_mul(
                st2[:, sb8, hl * 64:(hl + 1) * 64],
                acc[sb8 // 4][:, sb8 % 4, 0:DH],
                rec[:, sb8 // 4, sb8 % 4:sb8 % 4 + 1])
        if hl == 1:
            # PE-transpose st2 into concatT, 4 s-blocks per psum tile
            for g in range(2):
                pt = ps_proj.tile([128, 512], F16, tag="proj",
                                  name=f"ct_{sc}_{p}_{g}")
                for j in range(4):
                    nc.tensor.transpose(pt[:, j * 128:(j + 1) * 128],
                                        st2[:, g * 4 + j, :], ident[:])
                nc.vector.tensor_copy(
                    concatT[:, p, sc * 1024 + g * 512:
                            sc * 1024 + (g + 1) * 512], pt[:])
            if p == 0:
                oacc[sc] = outp.tile([128, SB8, 2, 512], F16, tag="oacc",
                                     name=f"oacc_{sc}")
            for j, (sb8, ec) in enumerate((s8, e) for s8 in range(SB8)
                                          for e in range(2)):
                injl.append((cur_iter[0] + 1 + (3 * j) // 2,
                             outproj_partial(sc, sb8, ec, p)))

    iters = [(sc, h, tb) for (sc, h) in UNITS for tb in range(TB)]
    next_unit = {UNITS[i]: UNITS[i + 1] for i in range(len(UNITS) - 1)}
    pend = deque()
    accs = {}

    def acc_init(sc, h):
        def emit():
            acc = (ps_acc.tile([128, 4, DH + 1], F32, tag="acca",
                               name=f"acca_{sc}_{h}"),
                   ps_acc.tile([128, 4, DH + 1], F32, tag="accb",
                               name=f"accb_{sc}_{h}"))
            nc.vector.memset(acc[0][:], 0.0)
            nc.vector.memset(acc[1][:], 0.0)
            accs[(sc, h)] = acc
        return once(("acci", sc, h), lambda: [], emit)

    def flush_one():
        # psum accumulation "zero regions" are whole 2KB banks, so the 4
        # per-s-block groups in one bank can't use the start bit: zero the
        # banks once with the DVE, then accumulate with start=False.
        ex, sc, h, tb = pend.popleft()
        v_chunk(tb, h // 2)()
        if tb == 0:
            acc_init(sc, h)()
        acc = accs[(sc, h)]
        for sb8 in range(SB8):
            nc.tensor.matmul(acc[sb8 // 4][:, sb8 % 4, :],
                             ex[:, sb8 * 128:(sb8 + 1) * 128],
                             vext[:, tb, h, :],
                             start=False, stop=False, skip_group_check=True)
        if tb == TB - 1:
            normalize(sc, h, accs.pop((sc, h)))
            if (sc, h) in next_unit:
                acc_init(*next_unit[(sc, h)])()

    for i, (sc, h, tb) in enumerate(iters):
        cur_iter[0] = i
        hl, p = h % 2, h // 2
        if tb == 0:
            # everything this (sc, h) unit's scores need, emitted now if the
            # due-queue hasn't gotten to it yet
            if p not in pair_w:
                pair_w[p] = load_pair_weights(p)
                emitted.add(("wld", p))
            for c in (2 * sc, 2 * sc + 1):
                qk_half(p, 0, c, 0)(); qk_half(p, 0, c, 1)()
        kc = tb // 4
        qk_half(p, 1, kc, 0)(); qk_half(p, 1, kc, 1)()
        scp = ps_slot.tile([128, 1024], F32, tag="slot", name=f"s_{i}")
        kblk = kT[hl * 64:(hl + 1) * 64, p, tb * 128:(tb + 1) * 128]
        for half in range(2):
            c0 = sc * 1024 + half * 512
            nc.tensor.matmul(scp[:, half * 512:(half + 1) * 512], kblk,
                             qT[hl * 64:(hl + 1) * 64, p, c0:c0 + 512],
                             start=True, stop=True)
        ex = exp_p.tile([128, 1024], F16, tag="ex", name=f"ex_{i}")
        nc.scalar.activation(ex[:], scp[:], Exp, scale=0.125)
        pend.append((ex, sc, h, tb))
        # unit 0 runs scores/exp only (backlog 16) so the early iterations
        # have budget for the v/x/qk chunk stream; drain the backlog at half
        # rate (a double-flush iteration stalls the ACT) back to lag-1
        if i < 16:
            cap = 17
        else:
            cap = max(2, 17 - (i - 16) // 3)
        nflush = 0
        while pend and len(pend) > cap - 1 and nflush < 2:
            flush_one()
            nflush += 1
        pump_inj(i)

    while pend:
        flush_one()
    while inj or injl:
        pump_inj(1 << 30, budget=4)

    if dbg is not None:
        dbg_qt, dbg_kt, dbg_vx, dbg_cc, dbg_xt = dbg
        nc.sync.dma_start(dbg_qt[:], qT[:])
        nc.sync.dma_start(dbg_kt[:], kT[:])
        nc.sync.dma_start(dbg_vx[:], vext[:])
        nc.sync.dma_start(dbg_cc[:], concatT[:])
        nc.sync.dma_start(dbg_xt[:], xT[:])


_CACHE = {}


def _build(debug=False):
    nc = bacc.Bacc("TRN2", target_bir_lowering=False, debug=False,
                   num_devices=N_CORES)
    x_d = nc.dram_tensor("x", [S, E], F16, kind="ExternalInput").ap()
    wq_d = nc.dram_tensor("wq", [E, JW], F16, kind="ExternalInput").ap()
    wk_d = nc.dram_tensor("wk", [E, JW], F16, kind="ExternalInput").ap()
    wv_d = nc.dram_tensor("wv", [E, JW], F16, kind="ExternalInput").ap()
    wo_d = nc.dram_tensor("wo", [JW, E], F16, kind="ExternalInput").ap()
    bqt_d = nc.dram_tensor("bqt", [128, 4], F32, kind="ExternalInput").ap()
    bkt_d = nc.dram_tensor("bkt", [128, 4], F32, kind="ExternalInput").ap()
    bv_d = nc.dram_tensor("bv", [1, JW], F32, kind="ExternalInput").ap()
    id_d = nc.dram_tensor("ident", [128, 128], F16, kind="ExternalInput").ap()
    out_d = nc.dram_tensor("out", [S, E], F16, kind="ExternalOutput").ap()
    aps = (x_d, wq_d, wk_d, wv_d, wo_d, bqt_d, bkt_d, bv_d, id_d, out_d)
    dbg = None
    if debug:
        dbg = (
            nc.dram_tensor("dbg_qt", [128, 4, S], F16,
                           kind="ExternalOutput").ap(),
            nc.dram_tensor("dbg_kt", [128, 4, S], F16,
                           kind="ExternalOutput").ap(),
            nc.dram_tensor("dbg_vx", [128, TB, HPC, DH + 1], F16,
                           kind="ExternalOutput").ap(),
            nc.dram_tensor("dbg_cc", [128, 4, S], F16,
                           kind="ExternalOutput").ap(),
            nc.dram_tensor("dbg_xt", [128, EB, S], F16,
                           kind="ExternalOutput").ap(),
        )
    with tile.TileContext(nc) as tc:
        with ExitStack() as ctx:
            _emit(tc, aps, ctx, dbg=dbg)
    nc.compile()
    return nc


def kernel(x, Wq, bq, Wk, bk, Wv, bv, Wo, bo):
    x = np.asarray(x, dtype=np.float32)
    Wq = np.asarray(Wq, dtype=np.float32)
    bq = np.asarray(bq, dtype=np.float32)
    Wk = np.asarray(Wk, dtype=np.float32)
    bk = np.asarray(bk, dtype=np.float32)
    Wv = np.asarray(Wv, dtype=np.float32)
    bv = np.asarray(bv, dtype=np.float32)
    Wo = np.asarray(Wo, dtype=np.float32)
    bo = np.asarray(bo, dtype=np.float32)

    if "nc" not in _CACHE:
        _CACHE["nc"] = _build()
    nc = _CACHE["nc"]

    WoT = np.ascontiguousarray(Wo.T)  # [f, e]
    in_maps = []
    for c in range(N_CORES):
        b, hh = c // 2, c % 2
        hs = slice(hh * HPC, (hh + 1) * HPC)
        in_maps.append({
            "x": np.ascontiguousarray(x[b]).astype(np.float16),
            "wq": np.ascontiguousarray(
                Wq[hs].transpose(1, 0, 2).reshape(E, JW)).astype(np.float16),
            "wk": np.ascontiguousarray(
                Wk[hs].transpose(1, 0, 2).reshape(E, JW)).astype(np.float16),
            "wv": np.ascontiguousarray(
                Wv[hs].transpose(1, 0, 2).reshape(E, JW)).astype(np.float16),
            "wo": np.ascontiguousarray(
                WoT[hh * JW:(hh + 1) * JW]).astype(np.float16),
            "bqt": np.ascontiguousarray(bq[hs].reshape(4, 128).T),
            "bkt": np.ascontiguousarray(bk[hs].reshape(4, 128).T),
            "bv": bv[hs].reshape(1, JW),
            "ident": np.eye(128, dtype=np.float16),
        })

    res = bass_utils.run_bass_kernel_spmd(nc, in_maps,
                                          core_ids=list(range(N_CORES)))
    out = np.empty((B, S, E), dtype=np.float32)
    for b in range(B):
        out[b] = (res.results[2 * b]["out"].astype(np.float32)
                  + res.results[2 * b + 1]["out"].astype(np.float32))
        out[b] += bo[None, :]
    return out


# revision 64
# speedup vs baseline: 1.2275x; 1.0167x over previous
"""Multi-head attention kernel for Trainium2, 8 NeuronCores.

Problem (hardcoded shapes): B=4, S=2048, E=1024, H=16, DH=64.
  q/k/v = einsum('bse,hed->bhsd', x, W{q,k,v}) + b{q,k,v}
  attn  = softmax(q k^T / sqrt(DH)) v
  out   = concat_heads(attn) @ Wo^T + bo

Sharding: core c -> (batch b = c//2, head-half hh = c%2, i.e. heads
8*hh..8*hh+7).  Each core computes a [S, E] partial of its batch's output;
the host sums the two partials per batch and adds bo.

Everything 2-byte (fp16) on the PE so every matmul runs at 1 col/cycle:
  xT   [e=128 x 8, s=2048]  f16, PE-transpose of DMA'd x s-blocks
  qT/kT [j=128, pair, s]    f16, Wq^T x + bias per pair (PE + DVE bias)
  vext [t, tb, h, 65]       f16, x Wv + bias, with a fused ones column
  scores [t=128, s=1024]    PSUM f32, two 512-col matmuls per group
  exp   [t=128, s=1024]     one ACT Exp(scale=1/8) instr -> SBUF f16
  attnV FLIPPED: out[s=128, 65] += exp-block (lhsT) @ vext[t,65] (rhs);
        65-col moving operand -> half the PE cost of the [65, s] orientation.
        PSUM accumulation uses DVE-zeroed banks + start=False because the
        hardware "zero region" is a whole bank (4 groups share each bank).
  normalize: DVE recip on the sums column + per-s-block tensor_scalar
  concatT via PE-transpose of the normalized [s, f] blocks
  outproj: per-pair partial matmuls accumulated in SBUF f16 by the DVE, so
        only the last pair's 16 partials sit in the tail

Scheduling: one global software-pipelined stream over (s-chunk, head,
t-block) groups; projection/outproj work is broken into ~0.4-0.9us chunks
and injected one per iteration from a due-time queue.  Every consumer
*pulls* (emit-once) its producer chunks first, so emission order is correct
by construction regardless of the due tuning.

NOTE: dma_start_transpose is avoided entirely — its completion semaphore
fires before the data lands (verified empirically: a matmul chasing the
transpose reads garbage), and concurrent xbar transposes corrupt.
"""

import os
import sys

for _p in ("/opt/trn_rl_repo", "/root/.axon_site/_ro/trn_rl_repo"):
    if os.path.isdir(_p) and _p not in sys.path:
        sys.path.insert(0, _p)
        break

from collections import deque
from contextlib import ExitStack

import numpy as np

import concourse.bass as bass
import concourse.tile as tile
import concourse.mybir as mybir
from concourse import bacc, bass_utils

B, S, E, H, DH = 4, 2048, 1024, 16, 64
HPC = 8           # heads per core
JW = HPC * DH     # 512, per-core qkv width
N_CORES = 8
EB = E // 128     # 8 e-blocks
TB = S // 128     # 16 t-blocks (also x s-blocks)
SB8 = 8           # s-blocks per 1024-wide s-chunk
F32 = mybir.dt.float32
F16 = mybir.dt.float16
Exp = mybir.ActivationFunctionType.Exp
ADD = mybir.AluOpType.add

# (sc, h) processing order: interleave the two s-chunks so each qk pair's
# projection window is ~4 units wide and outproj partials spread out.
UNITS = []
for hp in range(4):
    UNITS += [(0, 2 * hp), (0, 2 * hp + 1), (1, 2 * hp), (1, 2 * hp + 1)]


def _emit(tc, aps, ctx, dbg=None):
    nc = tc.nc
    x_d, wq_d, wk_d, wv_d, wo_d, bqt_d, bkt_d, bv_d, id_d, out_d = aps

    def pool(**kw):
        return ctx.enter_context(tc.tile_pool(**kw))

    const = pool(name="const", bufs=1)
    xs = pool(name="xs", bufs=1)
    xTp = pool(name="xT", bufs=1)
    vxp = pool(name="vext", bufs=1)
    wqk = pool(name="wqk", bufs=2)
    qkp = pool(name="qk", bufs=1)
    exp_p = pool(name="expS", bufs=17)
    st2p = pool(name="st2", bufs=2)
    recp = pool(name="rec", bufs=2)
    ccp = pool(name="concatT", bufs=1)
    outp = pool(name="outs", bufs=2)
    ps_slot = pool(name="ps_slot", bufs=2, space="PSUM")
    ps_acc = pool(name="ps_acc", bufs=1, space="PSUM")
    ps_proj = pool(name="ps_proj", bufs=1, space="PSUM")

    # ---- DMAs, in consumption order, all on the sync queue ----
    wq_r = wq_d.rearrange("(eb pp) j -> pp eb j", pp=128)
    wk_r = wk_d.rearrange("(eb pp) j -> pp eb j", pp=128)

    def load_pair_weights(p):
        wk_t = wqk.tile([128, EB, 128], F16, tag="wk", name=f"wk{p}")
        nc.sync.dma_start(wk_t[:], wk_r[:, :, p * 128:(p + 1) * 128])
        wq_t = wqk.tile([128, EB, 128], F16, tag="wq", name=f"wq{p}")
        nc.sync.dma_start(wq_t[:], wq_r[:, :, p * 128:(p + 1) * 128])
        return wq_t, wk_t

    wv_sb = const.tile([128, EB, JW], F16)
    wo_sb = const.tile([128, 4, E], F16)

    # x staged in four 4-s-block group DMAs (one instruction each), with the
    # weight loads slotted between them in consumption order
    x_groups = {}
    x_r = x_d.rearrange("(g p) e -> p g e", p=128)

    def dma_xg(g):
        x_t = xs.tile([128, 4, E], F16, tag="x_t", name=f"x_g_{g}")
        nc.sync.dma_start(x_t[:], x_r[:, 4 * g:4 * g + 4, :])
        x_groups[g] = x_t

    dma_xg(0)
    ident = const.tile([128, 128], F16)
    nc.sync.dma_start(ident[:], id_d[:])
    bq_sb = const.tile([128, 4], F32)
    nc.sync.dma_start(bq_sb[:], bqt_d[:])
    bk_sb = const.tile([128, 4], F32)
    nc.sync.dma_start(bk_sb[:], bkt_d[:])
    bv1 = const.tile([1, JW], F32)
    nc.sync.dma_start(bv1[:], bv_d[:])
    pair_w = {0: load_pair_weights(0)}
    dma_xg(1)
    dma_xg(2)
    dma_xg(3)
    nc.sync.dma_start(wv_sb[:], wv_d.rearrange("(eb p) j -> p eb j", p=128))
    nc.sync.dma_start(wo_sb[:], wo_d.rearrange("(fb p) e -> p fb e", p=128))

    # PE p-state warmup: the tensor engine ramps to full clock only after
    # ~3us of sustained use, and the ramp clock starts at the first busy
    # period.  Run throwaway matmuls from t~0.3us so the real prefix work
    # (from ~4.5us, when x group 0 lands) runs at full speed.
    warm = const.tile([128, 512], F16)
    nc.vector.memset(warm[:], 0.25)
    for i in range(18):
        pw = ps_slot.tile([128, 512], F32, tag="slot", name=f"warm{i}")
        nc.tensor.matmul(pw[:], warm[:, 0:128], warm[:], start=True, stop=True)

    bvb = const.tile([128, JW], F32)
    nc.gpsimd.partition_broadcast(bvb[:], bv1[:])

    vext = vxp.tile([128, TB, HPC, DH + 1], F16)
    nc.gpsimd.memset(vext[:, :, :, DH:DH + 1], 1.0)

    qT = qkp.tile([128, 4, S], F16, tag="qT")
    kT = qkp.tile([128, 4, S], F16, tag="kT")
    xT = xTp.tile([128, EB, S], F16)
    concatT = ccp.tile([128, 4, S], F16)

    # ---- emit-once chunk machinery: consumers pull producers ----
    emitted = set()

    def once(key, deps, fn):
        def run():
            if key in emitted:
                return
            emitted.add(key)
            for d in deps():
                d()
            fn()
        return run

    def x_chunk(sb):
        """PE-transpose x s-block sb into xT.  Uses the score-slot psum pool
        (2 buffers, idle during the prefix) so consecutive chunks double-
        buffer instead of serializing on a single-bank WAR chain."""
        def emit():
            x_t = x_groups[sb // 4][:, sb % 4, :]
            pt = ps_slot.tile([128, 1024], F16, tag="slot", name=f"pt_{sb}")
            for eb in range(EB):
                nc.tensor.transpose(pt[:, eb * 128:(eb + 1) * 128],
                                    x_t[:, eb * 128:(eb + 1) * 128],
                                    ident[:])
            nc.vector.tensor_copy(
                xT[:, :, sb * 128:(sb + 1) * 128],
                pt[:].rearrange("p (e s) -> p e s", e=8))
        return once(("x", sb), lambda: [], emit)

    _qk_ps = {}
    _qk_open = [None]

    def qk_half(p, qk, c, half):
        """Half (4 e-blocks) of one 512-col q/k projection chunk.  The "pq"
        psum pool has one buffer, so before opening a new chunk any other
        half-open chunk is closed first."""
        def emit():
            if half == 0 and _qk_open[0] is not None:
                op, oqk, oc = _qk_open[0]
                qk_half(op, oqk, oc, 1)()
            wq_t, wk_t = pair_w[p]
            w_t, dst, b_sb = ((wq_t, qT, bq_sb) if qk == 0 else
                              (wk_t, kT, bk_sb))
            if half == 0:
                pq = ps_proj.tile([128, 512], F32, tag="pq",
                                  name=f"pq_{p}_{qk}_{c}")
                _qk_ps[(p, qk, c)] = pq
                _qk_open[0] = (p, qk, c)
            else:
                pq = _qk_ps.pop((p, qk, c))
                _qk_open[0] = None
            for q in range(4):
                eb = half * 4 + q
                nc.tensor.matmul(pq[:], w_t[:, eb, :],
                                 xT[:, eb, c * 512:(c + 1) * 512],
                                 start=(eb == 0), stop=(eb == EB - 1))
            if half == 1:
                nc.vector.tensor_scalar_add(
                    dst[:, p, c * 512:(c + 1) * 512], pq[:], b_sb[:, p:p + 1])

        def deps():
            d = [x_chunk(sb) for sb in range(4 * c, 4 * c + 4)]
            if half == 1:
                d.append(qk_half(p, qk, c, 0))
            return d
        return once(("qk", p, qk, c, half), deps, emit)

    def v_chunk(tb, hp):
        """Project v for t-block tb, head pair hp (2 heads, 128 cols).
        Alternates between the proj and (when no q/k chunk is half-open)
        pq psum banks so consecutive chunks double-buffer."""
        def emit():
            pv = ps_proj.tile([128, 128], F32, tag="proj",
                              name=f"pv_{tb}_{hp}")
            for eb in range(EB):
                nc.tensor.matmul(pv[:], xT[:, eb, tb * 128:(tb + 1) * 128],
                                 wv_sb[:, eb, hp * 128:(hp + 1) * 128],
                                 start=(eb == 0), stop=(eb == EB - 1))
            nc.vector.tensor_tensor(
                vext[:, tb, 2 * hp:2 * hp + 2, 0:DH],
                pv[:].rearrange("p (h d) -> p h d", h=2),
                bvb[:, hp * 128:(hp + 1) * 128].rearrange(
                    "p (h d) -> p h d", h=2), ADD)
        return once(("v", tb, hp), lambda: [x_chunk(tb)], emit)

    oacc = {}

    def outproj_partial(sc, sb8, ec, p):
        def emit():
            # the last pair of the last s-chunk is the kernel tail: ping-pong
            # through the freed "pq" bank, and route half the accumulates
            # through the idle ACT engine (PE identity-matmul adds oacc into
            # the psum group, ACT copies it out) so DVE and ACT split the work
            tail = (p == 3 and sc == 1)
            tag = "pq" if (tail and (sb8 + ec) % 2) else "proj"
            po = ps_proj.tile([128, 512], F32, tag=tag,
                              name=f"po_{sc}_{sb8}_{ec}_{p}")
            oa = oacc[sc][:, sb8, ec, :]
            act_lane = tail and (sb8 + ec) % 2
            nc.tensor.matmul(po[:],
                             concatT[:, p, sc * 1024 + sb8 * 128:
                                     sc * 1024 + (sb8 + 1) * 128],
                             wo_sb[:, p, ec * 512:(ec + 1) * 512],
                             start=True, stop=not act_lane)
            if act_lane:
                nc.tensor.matmul(po[:], ident[:], oa, start=False, stop=True)
                nc.scalar.copy(oa, po[:])
            elif p == 0:
                nc.vector.tensor_copy(oa, po[:])
            else:
                nc.vector.tensor_tensor(oa, po[:], oa, ADD)
            if p == 3 and ec == 1:
                r0 = sc * 1024 + sb8 * 128
                nc.sync.dma_start(out_d[r0:r0 + 128, :],
                                  oacc[sc][:, sb8, :, :])
        return once(("op", sc, sb8, ec, p), lambda: [], emit)

    # ---- prefix PE work: x transposes chase the x DMAs; pair-0 k/q chunks
    # chase the transposes e-block by e-block ----
    for sb in range(4):
        x_chunk(sb)()
    qk_half(0, 1, 0, 0)(); qk_half(0, 1, 0, 1)()
    qk_half(0, 0, 0, 0)(); qk_half(0, 0, 0, 1)()
    for sb in range(4, 8):
        x_chunk(sb)()
    qk_half(0, 0, 1, 0)(); qk_half(0, 0, 1, 1)()

    # ---- due-scheduled injection queue (performance tuning only) ----
    inj = []
    injl = deque()

    def qk_halves(p, qk, c, due):
        inj.append((due, qk_half(p, qk, c, 0)))
        inj.append((due, qk_half(p, qk, c, 1)))

    for sb in range(8, TB):
        inj.append((sb - 4, x_chunk(sb)))
    qk_halves(0, 1, 1, 2)
    qk_halves(0, 1, 2, 8)
    qk_halves(0, 1, 3, 12)
    qk_halves(0, 0, 2, 17)
    qk_halves(0, 0, 3, 21)
    # unit 0's attnV is deferred ~16 iters, so vext t-blocks are only needed
    # from iter ~16+tb
    for tb in range(1, TB):
        inj.append((11 + tb, v_chunk(tb, 0)))
    for hp, base in ((1, 38), (2, 98), (3, 162)):
        for tb in range(TB):
            inj.append((base + tb, v_chunk(tb, hp)))
    for p, base in ((1, 34), (2, 94), (3, 158)):
        def mk_load(p=p):
            def emit():
                pair_w[p] = load_pair_weights(p)
            return once(("wld", p), lambda: [], emit)
        inj.append((base - 2, mk_load()))
        seq = [(1, 0), (0, 0), (0, 1), (1, 1), (1, 2), (1, 3)]
        for i, (qk, c) in enumerate(seq):
            qk_halves(p, qk, c, base + 3 * i)
        qk_halves(p, 0, 2, base + 24)
        qk_halves(p, 0, 3, base + 28)
    inj.sort(key=lambda t: t[0])
    inj = deque(inj)

    def pump_inj(i, budget=1):
        for _ in range(budget):
            src = None
            if inj and injl:
                src = inj if inj[0][0] <= injl[0][0] else injl
            elif inj or injl:
                src = inj or injl
            if src is None:
                return
            src.popleft()[1]()

    # ---- the main software-pipelined stream ----
    st2s = {}
    cur_iter = [0]

    def normalize(sc, h, acc):
        p, hl = h // 2, h % 2
        rec = recp.tile([128, 2, 4], F32, tag="rec", name=f"rec_{sc}_{h}")
        nc.vector.reciprocal(rec[:, 0, :], acc[0][:, :, DH:DH + 1])
        nc.vector.reciprocal(rec[:, 1, :], acc[1][:, :, DH:DH + 1])
        if hl == 0:
            st2s[(p, sc)] = st2p.tile([128, SB8, 128], F16, tag="st2",
                                      name=f"st2_{p}_{sc}")
        st2 = st2s[(p, sc)]
        for sb8 in range(SB8):
            nc.vector.tensor_scalar_mul(
                st2[:, sb8, hl * 64:(hl + 1) * 64],
                acc[sb8 // 4][:, sb8 % 4, 0:DH],
                rec[:, sb8 // 4, sb8 % 4:sb8 % 4 + 1])
        if hl == 1:
            # PE-transpose st2 into concatT, 4 s-blocks per psum tile
            for g in range(2):
                pt = ps_proj.tile([128, 512], F16, tag="proj",
                                  name=f"ct_{sc}_{p}_{g}")
                for j in range(4):
                    nc.tensor.transpose(pt[:, j * 128:(j + 1) * 128],
                                        st2[:, g * 4 + j, :], ident[:])
                nc.vector.tensor_copy(
                    concatT[:, p, sc * 1024 + g * 512:
                            sc * 1024 + (g + 1) * 512], pt[:])
            if p == 0:
                oacc[sc] = outp.tile([128, SB8, 2, 512], F16, tag="oacc",
                                     name=f"oacc_{sc}")
            for j, (sb8, ec) in enumerate((s8, e) for s8 in range(SB8)
                                          for e in range(2)):
                injl.append((cur_iter[0] + 1 + (3 * j) // 2,
                             outproj_partial(sc, sb8, ec, p)))

    iters = [(sc, h, tb) for (sc, h) in UNITS for tb in range(TB)]
    next_unit = {UNITS[i]: UNITS[i + 1] for i in range(len(UNITS) - 1)}
    pend = deque()
    accs = {}

    def acc_init(sc, h):
        def emit():
            acc = (ps_acc.tile([128, 4, DH + 1], F32, tag="acca",
                               name=f"acca_{sc}_{h}"),
                   ps_acc.tile([128, 4, DH + 1], F32, tag="accb",
                               name=f"accb_{sc}_{h}"))
            nc.vector.memset(acc[0][:], 0.0)
            nc.vector.memset(acc[1][:], 0.0)
            accs[(sc, h)] = acc
        return once(("acci", sc, h), lambda: [], emit)

    def flush_one():
        # psum accumulation "zero regions" are whole 2KB banks, so the 4
        # per-s-block groups in one bank can't use the start bit: zero the
        # banks once with the DVE, then accumulate with start=False.
        ex, sc, h, tb = pend.popleft()
        v_chunk(tb, h // 2)()
        if tb == 0:
            acc_init(sc, h)()
        acc = accs[(sc, h)]
        for sb8 in range(SB8):
            nc.tensor.matmul(acc[sb8 // 4][:, sb8 % 4, :],
                             ex[:, sb8 * 128:(sb8 + 1) * 128],
                             vext[:, tb, h, :],
                             start=False, stop=False, skip_group_check=True)
        if tb == TB - 1:
            normalize(sc, h, accs.pop((sc, h)))
            if (sc, h) in next_unit:
                acc_init(*next_unit[(sc, h)])()

    def emit_score(i):
        sc, h, tb = iters[i]
        hl, p = h % 2, h // 2
        if tb == 0:
            # everything this (sc, h) unit's scores need, emitted now if the
            # due-queue hasn't gotten to it yet
            if p not in pair_w:
                pair_w[p] = load_pair_weights(p)
                emitted.add(("wld", p))
            for c in (2 * sc, 2 * sc + 1):
                qk_half(p, 0, c, 0)(); qk_half(p, 0, c, 1)()
        kc = tb // 4
        qk_half(p, 1, kc, 0)(); qk_half(p, 1, kc, 1)()
        scp = ps_slot.tile([128, 1024], F32, tag="slot", name=f"s_{i}")
        kblk = kT[hl * 64:(hl + 1) * 64, p, tb * 128:(tb + 1) * 128]
        for half in range(2):
            c0 = sc * 1024 + half * 512
            nc.tensor.matmul(scp[:, half * 512:(half + 1) * 512], kblk,
                             qT[hl * 64:(hl + 1) * 64, p, c0:c0 + 512],
                             start=True, stop=True)
        return scp

    # scores are emitted one iteration ahead of their exp, so the PE reaches
    # the next score matmul the moment its pong slot frees instead of sitting
    # behind the attnV/chunk work of the current iteration
    scps = {0: emit_score(0)}
    for i, (sc, h, tb) in enumerate(iters):
        cur_iter[0] = i
        ex = exp_p.tile([128, 1024], F16, tag="ex", name=f"ex_{i}")
        nc.scalar.activation(ex[:], scps.pop(i)[:], Exp, scale=0.125)
        if i + 1 < len(iters):
            scps[i + 1] = emit_score(i + 1)
        pend.append((ex, sc, h, tb))
        # unit 0 runs scores/exp only (backlog 16) so the early iterations
        # have budget for the v/x/qk chunk stream; drain the backlog at half
        # rate (a double-flush iteration stalls the ACT) back to lag-1
        if i < 16:
            cap = 17
        else:
            cap = max(2, 17 - (i - 16) // 3)
        nflush = 0
        while pend and len(pend) > cap - 1 and nflush < 2:
            flush_one()
            nflush += 1
        pump_inj(i)

    while pend:
        flush_one()
    while inj or injl:
        pump_inj(1 << 30, budget=4)

    if dbg is not None:
        dbg_qt, dbg_kt, dbg_vx, dbg_cc, dbg_xt = dbg
        nc.sync.dma_start(dbg_qt[:], qT[:])
        nc.sync.dma_start(dbg_kt[:], kT[:])
        nc.sync.dma_start(dbg_vx[:], vext[:])
        nc.sync.dma_start(dbg_cc[:], concatT[:])
        nc.sync.dma_start(dbg_xt[:], xT[:])


_CACHE = {}


def _build(debug=False):
    nc = bacc.Bacc("TRN2", target_bir_lowering=False, debug=False,
                   num_devices=N_CORES)
    x_d = nc.dram_tensor("x", [S, E], F16, kind="ExternalInput").ap()
    wq_d = nc.dram_tensor("wq", [E, JW], F16, kind="ExternalInput").ap()
    wk_d = nc.dram_tensor("wk", [E, JW], F16, kind="ExternalInput").ap()
    wv_d = nc.dram_tensor("wv", [E, JW], F16, kind="ExternalInput").ap()
    wo_d = nc.dram_tensor("wo", [JW, E], F16, kind="ExternalInput").ap()
    bqt_d = nc.dram_tensor("bqt", [128, 4], F32, kind="ExternalInput").ap()
    bkt_d = nc.dram_tensor("bkt", [128, 4], F32, kind="ExternalInput").ap()
    bv_d = nc.dram_tensor("bv", [1, JW], F32, kind="ExternalInput").ap()
    id_d = nc.dram_tensor("ident", [128, 128], F16, kind="ExternalInput").ap()
    out_d = nc.dram_tensor("out", [S, E], F16, kind="ExternalOutput").ap()
    aps = (x_d, wq_d, wk_d, wv_d, wo_d, bqt_d, bkt_d, bv_d, id_d, out_d)
    dbg = None
    if debug:
        dbg = (
            nc.dram_tensor("dbg_qt", [128, 4, S], F16,
                           kind="ExternalOutput").ap(),
            nc.dram_tensor("dbg_kt", [128, 4, S], F16,
                           kind="ExternalOutput").ap(),
            nc.dram_tensor("dbg_vx", [128, TB, HPC, DH + 1], F16,
                           kind="ExternalOutput").ap(),
            nc.dram_tensor("dbg_cc", [128, 4, S], F16,
                           kind="ExternalOutput").ap(),
            nc.dram_tensor("dbg_xt", [128, EB, S], F16,
                           kind="ExternalOutput").ap(),
        )
    with tile.TileContext(nc) as tc:
        with ExitStack() as ctx:
            _emit(tc, aps, ctx, dbg=dbg)
    nc.compile()
    return nc


def kernel(x, Wq, bq, Wk, bk, Wv, bv, Wo, bo):
    x = np.asarray(x, dtype=np.float32)
    Wq = np.asarray(Wq, dtype=np.float32)
    bq = np.asarray(bq, dtype=np.float32)
    Wk = np.asarray(Wk, dtype=np.float32)
    bk = np.asarray(bk, dtype=np.float32)
    Wv = np.asarray(Wv, dtype=np.float32)
    bv = np.asarray(bv, dtype=np.float32)
    Wo = np.asarray(Wo, dtype=np.float32)
    bo = np.asarray(bo, dtype=np.float32)

    if "nc" not in _CACHE:
        _CACHE["nc"] = _build()
    nc = _CACHE["nc"]

    WoT = np.ascontiguousarray(Wo.T)  # [f, e]
    in_maps = []
    for c in range(N_CORES):
        b, hh = c // 2, c % 2
        hs = slice(hh * HPC, (hh + 1) * HPC)
        in_maps.append({
            "x": np.ascontiguousarray(x[b]).astype(np.float16),
            "wq": np.ascontiguousarray(
                Wq[hs].transpose(1, 0, 2).reshape(E, JW)).astype(np.float16),
            "wk": np.ascontiguousarray(
                Wk[hs].transpose(1, 0, 2).reshape(E, JW)).astype(np.float16),
            "wv": np.ascontiguousarray(
                Wv[hs].transpose(1, 0, 2).reshape(E, JW)).astype(np.float16),
            "wo": np.ascontiguousarray(
                WoT[hh * JW:(hh + 1) * JW]).astype(np.float16),
            "bqt": np.ascontiguousarray(bq[hs].reshape(4, 128).T),
            "bkt": np.ascontiguousarray(bk[hs].reshape(4, 128).T),
            "bv": bv[hs].reshape(1, JW),
            "ident": np.eye(128, dtype=np.float16),
        })

    res = bass_utils.run_bass_kernel_spmd(nc, in_maps,
                                          core_ids=list(range(N_CORES)))
    out = np.empty((B, S, E), dtype=np.float32)
    for b in range(B):
        out[b] = (res.results[2 * b]["out"].astype(np.float32)
                  + res.results[2 * b + 1]["out"].astype(np.float32))
        out[b] += bo[None, :]
    return out


# revision 65
# speedup vs baseline: 1.2280x; 1.0004x over previous
"""Multi-head attention kernel for Trainium2, 8 NeuronCores.

Problem (hardcoded shapes): B=4, S=2048, E=1024, H=16, DH=64.
  q/k/v = einsum('bse,hed->bhsd', x, W{q,k,v}) + b{q,k,v}
  attn  = softmax(q k^T / sqrt(DH)) v
  out   = concat_heads(attn) @ Wo^T + bo

Sharding: core c -> (batch b = c//2, head-half hh = c%2, i.e. heads
8*hh..8*hh+7).  Each core computes a [S, E] partial of its batch's output;
the host sums the two partials per batch and adds bo.

Everything 2-byte (fp16) on the PE so every matmul runs at 1 col/cycle:
  xT   [e=128 x 8, s=2048]  f16, PE-transpose of DMA'd x s-blocks
  qT/kT [j=128, pair, s]    f16, Wq^T x + bias per pair (PE + DVE bias)
  vext [t, tb, h, 65]       f16, x Wv + bias, with a fused ones column
  scores [t=128, s=1024]    PSUM f32, two 512-col matmuls per group
  exp   [t=128, s=1024]     one ACT Exp(scale=1/8) instr -> SBUF f16
  attnV FLIPPED: out[s=128, 65] += exp-block (lhsT) @ vext[t,65] (rhs);
        65-col moving operand -> half the PE cost of the [65, s] orientation.
        PSUM accumulation uses DVE-zeroed banks + start=False because the
        hardware "zero region" is a whole bank (4 groups share each bank).
  normalize: DVE recip on the sums column + per-s-block tensor_scalar
  concatT via PE-transpose of the normalized [s, f] blocks
  outproj: per-pair partial matmuls accumulated in SBUF f16 by the DVE, so
        only the last pair's 16 partials sit in the tail

Scheduling: one global software-pipelined stream over (s-chunk, head,
t-block) groups; projection/outproj work is broken into ~0.4-0.9us chunks
and injected one per iteration from a due-time queue.  Every consumer
*pulls* (emit-once) its producer chunks first, so emission order is correct
by construction regardless of the due tuning.

NOTE: dma_start_transpose is avoided entirely — its completion semaphore
fires before the data lands (verified empirically: a matmul chasing the
transpose reads garbage), and concurrent xbar transposes corrupt.
"""

import os
import sys

for _p in ("/opt/trn_rl_repo", "/root/.axon_site/_ro/trn_rl_repo"):
    if os.path.isdir(_p) and _p not in sys.path:
        sys.path.insert(0, _p)
        break

from collections import deque
from contextlib import ExitStack

import numpy as np

import concourse.bass as bass
import concourse.tile as tile
import concourse.mybir as mybir
from concourse import bacc, bass_utils

B, S, E, H, DH = 4, 2048, 1024, 16, 64
HPC = 8           # heads per core
JW = HPC * DH     # 512, per-core qkv width
N_CORES = 8
EB = E // 128     # 8 e-blocks
TB = S // 128     # 16 t-blocks (also x s-blocks)
SB8 = 8           # s-blocks per 1024-wide s-chunk
F32 = mybir.dt.float32
F16 = mybir.dt.float16
Exp = mybir.ActivationFunctionType.Exp
ADD = mybir.AluOpType.add

# (sc, h) processing order: interleave the two s-chunks so each qk pair's
# projection window is ~4 units wide and outproj partials spread out.
UNITS = []
for hp in range(4):
    UNITS += [(0, 2 * hp), (0, 2 * hp + 1), (1, 2 * hp), (1, 2 * hp + 1)]


def _emit(tc, aps, ctx, dbg=None):
    nc = tc.nc
    x_d, wq_d, wk_d, wv_d, wo_d, bqt_d, bkt_d, bv_d, id_d, out_d = aps

    def pool(**kw):
        return ctx.enter_context(tc.tile_pool(**kw))

    const = pool(name="const", bufs=1)
    xs = pool(name="xs", bufs=1)
    xTp = pool(name="xT", bufs=1)
    vxp = pool(name="vext", bufs=1)
    wqk = pool(name="wqk", bufs=2)
    qkp = pool(name="qk", bufs=1)
    exp_p = pool(name="expS", bufs=17)
    st2p = pool(name="st2", bufs=2)
    recp = pool(name="rec", bufs=2)
    ccp = pool(name="concatT", bufs=1)
    outp = pool(name="outs", bufs=2)
    ps_slot = pool(name="ps_slot", bufs=2, space="PSUM")
    ps_acc = pool(name="ps_acc", bufs=1, space="PSUM")
    ps_proj = pool(name="ps_proj", bufs=1, space="PSUM")

    # ---- DMAs, in consumption order, all on the sync queue ----
    wq_r = wq_d.rearrange("(eb pp) j -> pp eb j", pp=128)
    wk_r = wk_d.rearrange("(eb pp) j -> pp eb j", pp=128)

    def load_pair_weights(p):
        wk_t = wqk.tile([128, EB, 128], F16, tag="wk", name=f"wk{p}")
        nc.sync.dma_start(wk_t[:], wk_r[:, :, p * 128:(p + 1) * 128])
        wq_t = wqk.tile([128, EB, 128], F16, tag="wq", name=f"wq{p}")
        nc.sync.dma_start(wq_t[:], wq_r[:, :, p * 128:(p + 1) * 128])
        return wq_t, wk_t

    wv_sb = const.tile([128, EB, JW], F16)
    wo_sb = const.tile([128, 4, E], F16)

    # x staged in four 4-s-block group DMAs (one instruction each), with the
    # weight loads slotted between them in consumption order
    x_groups = {}
    x_r = x_d.rearrange("(g p) e -> p g e", p=128)

    def dma_xg(g):
        x_t = xs.tile([128, 4, E], F16, tag="x_t", name=f"x_g_{g}")
        nc.sync.dma_start(x_t[:], x_r[:, 4 * g:4 * g + 4, :])
        x_groups[g] = x_t

    dma_xg(0)
    ident = const.tile([128, 128], F16)
    nc.sync.dma_start(ident[:], id_d[:])
    bq_sb = const.tile([128, 4], F32)
    nc.sync.dma_start(bq_sb[:], bqt_d[:])
    bk_sb = const.tile([128, 4], F32)
    nc.sync.dma_start(bk_sb[:], bkt_d[:])
    bv1 = const.tile([1, JW], F32)
    nc.sync.dma_start(bv1[:], bv_d[:])
    pair_w = {0: load_pair_weights(0)}
    dma_xg(1)
    dma_xg(2)
    dma_xg(3)
    nc.sync.dma_start(wv_sb[:], wv_d.rearrange("(eb p) j -> p eb j", p=128))
    nc.sync.dma_start(wo_sb[:], wo_d.rearrange("(fb p) e -> p fb e", p=128))

    # PE p-state warmup: the tensor engine ramps to full clock only after
    # ~3us of sustained use, and the ramp clock starts at the first busy
    # period.  Run throwaway matmuls from t~0.3us so the real prefix work
    # (from ~4.5us, when x group 0 lands) runs at full speed.
    warm = const.tile([128, 512], F16)
    nc.vector.memset(warm[:], 0.25)
    for i in range(18):
        pw = ps_slot.tile([128, 512], F32, tag="slot", name=f"warm{i}")
        nc.tensor.matmul(pw[:], warm[:, 0:128], warm[:], start=True, stop=True)

    bvb = const.tile([128, JW], F32)
    nc.gpsimd.partition_broadcast(bvb[:], bv1[:])

    vext = vxp.tile([128, TB, HPC, DH + 1], F16)
    nc.gpsimd.memset(vext[:, :, :, DH:DH + 1], 1.0)

    qT = qkp.tile([128, 4, S], F16, tag="qT")
    kT = qkp.tile([128, 4, S], F16, tag="kT")
    xT = xTp.tile([128, EB, S], F16)
    concatT = ccp.tile([128, 4, S], F16)

    # ---- emit-once chunk machinery: consumers pull producers ----
    emitted = set()

    def once(key, deps, fn):
        def run():
            if key in emitted:
                return
            emitted.add(key)
            for d in deps():
                d()
            fn()
        return run

    def x_chunk(sb):
        """PE-transpose x s-block sb into xT.  Uses the score-slot psum pool
        (2 buffers, idle during the prefix) so consecutive chunks double-
        buffer instead of serializing on a single-bank WAR chain."""
        def emit():
            x_t = x_groups[sb // 4][:, sb % 4, :]
            pt = ps_slot.tile([128, 1024], F16, tag="slot", name=f"pt_{sb}")
            for eb in range(EB):
                nc.tensor.transpose(pt[:, eb * 128:(eb + 1) * 128],
                                    x_t[:, eb * 128:(eb + 1) * 128],
                                    ident[:])
            nc.vector.tensor_copy(
                xT[:, :, sb * 128:(sb + 1) * 128],
                pt[:].rearrange("p (e s) -> p e s", e=8))
        return once(("x", sb), lambda: [], emit)

    _qk_ps = {}
    _qk_open = [None]
    _qk_use_slot = [True]  # prefix chunks double-buffer via the slot pool

    def qk_half(p, qk, c, half):
        """Half (4 e-blocks) of one 512-col q/k projection chunk.  The "pq"
        psum pool has one buffer, so before opening a new chunk any other
        half-open chunk is closed first."""
        def emit():
            if half == 0 and _qk_open[0] is not None:
                op, oqk, oc = _qk_open[0]
                qk_half(op, oqk, oc, 1)()
            wq_t, wk_t = pair_w[p]
            w_t, dst, b_sb = ((wq_t, qT, bq_sb) if qk == 0 else
                              (wk_t, kT, bk_sb))
            if half == 0:
                if _qk_use_slot[0]:
                    pq = ps_slot.tile([128, 512], F32, tag="slot",
                                      name=f"pq_{p}_{qk}_{c}")
                else:
                    pq = ps_proj.tile([128, 512], F32, tag="pq",
                                      name=f"pq_{p}_{qk}_{c}")
                    _qk_open[0] = (p, qk, c)
                _qk_ps[(p, qk, c)] = pq
            else:
                pq = _qk_ps.pop((p, qk, c))
                if not _qk_use_slot[0]:
                    _qk_open[0] = None
            for q in range(4):
                eb = half * 4 + q
                nc.tensor.matmul(pq[:], w_t[:, eb, :],
                                 xT[:, eb, c * 512:(c + 1) * 512],
                                 start=(eb == 0), stop=(eb == EB - 1))
            if half == 1:
                nc.vector.tensor_scalar_add(
                    dst[:, p, c * 512:(c + 1) * 512], pq[:], b_sb[:, p:p + 1])

        def deps():
            d = [x_chunk(sb) for sb in range(4 * c, 4 * c + 4)]
            if half == 1:
                d.append(qk_half(p, qk, c, 0))
            return d
        return once(("qk", p, qk, c, half), deps, emit)

    def v_chunk(tb, hp):
        """Project v for t-block tb, head pair hp (2 heads, 128 cols).
        Alternates between the proj and (when no q/k chunk is half-open)
        pq psum banks so consecutive chunks double-buffer."""
        def emit():
            pv = ps_proj.tile([128, 128], F32, tag="proj",
                              name=f"pv_{tb}_{hp}")
            for eb in range(EB):
                nc.tensor.matmul(pv[:], xT[:, eb, tb * 128:(tb + 1) * 128],
                                 wv_sb[:, eb, hp * 128:(hp + 1) * 128],
                                 start=(eb == 0), stop=(eb == EB - 1))
            nc.vector.tensor_tensor(
                vext[:, tb, 2 * hp:2 * hp + 2, 0:DH],
                pv[:].rearrange("p (h d) -> p h d", h=2),
                bvb[:, hp * 128:(hp + 1) * 128].rearrange(
                    "p (h d) -> p h d", h=2), ADD)
        return once(("v", tb, hp), lambda: [x_chunk(tb)], emit)

    oacc = {}

    def outproj_partial(sc, sb8, ec, p):
        def emit():
            # the last pair of the last s-chunk is the kernel tail: ping-pong
            # through the freed "pq" bank, and route half the accumulates
            # through the idle ACT engine (PE identity-matmul adds oacc into
            # the psum group, ACT copies it out) so DVE and ACT split the work
            tail = (p == 3 and sc == 1)
            tag = "pq" if (tail and (sb8 + ec) % 2) else "proj"
            po = ps_proj.tile([128, 512], F32, tag=tag,
                              name=f"po_{sc}_{sb8}_{ec}_{p}")
            oa = oacc[sc][:, sb8, ec, :]
            act_lane = tail and (sb8 + ec) % 2
            nc.tensor.matmul(po[:],
                             concatT[:, p, sc * 1024 + sb8 * 128:
                                     sc * 1024 + (sb8 + 1) * 128],
                             wo_sb[:, p, ec * 512:(ec + 1) * 512],
                             start=True, stop=not act_lane)
            if act_lane:
                nc.tensor.matmul(po[:], ident[:], oa, start=False, stop=True)
                nc.scalar.copy(oa, po[:])
            elif p == 0:
                nc.vector.tensor_copy(oa, po[:])
            else:
                nc.vector.tensor_tensor(oa, po[:], oa, ADD)
            if p == 3 and ec == 1:
                r0 = sc * 1024 + sb8 * 128
                nc.sync.dma_start(out_d[r0:r0 + 128, :],
                                  oacc[sc][:, sb8, :, :])
        return once(("op", sc, sb8, ec, p), lambda: [], emit)

    # ---- prefix PE work: x transposes chase the x DMAs; pair-0 k/q chunks
    # chase the transposes e-block by e-block ----
    for sb in range(4):
        x_chunk(sb)()
    qk_half(0, 1, 0, 0)(); qk_half(0, 1, 0, 1)()
    qk_half(0, 0, 0, 0)(); qk_half(0, 0, 0, 1)()
    for sb in range(4, 8):
        x_chunk(sb)()
    qk_half(0, 0, 1, 0)(); qk_half(0, 0, 1, 1)()
    _qk_use_slot[0] = False

    # ---- due-scheduled injection queue (performance tuning only) ----
    inj = []
    injl = deque()

    def qk_halves(p, qk, c, due):
        inj.append((due, qk_half(p, qk, c, 0)))
        inj.append((due, qk_half(p, qk, c, 1)))

    for sb in range(8, TB):
        inj.append((sb - 4, x_chunk(sb)))
    qk_halves(0, 1, 1, 2)
    qk_halves(0, 1, 2, 8)
    qk_halves(0, 1, 3, 12)
    qk_halves(0, 0, 2, 17)
    qk_halves(0, 0, 3, 21)
    # unit 0's attnV is deferred ~16 iters, so vext t-blocks are only needed
    # from iter ~16+tb
    for tb in range(1, TB):
        inj.append((11 + tb, v_chunk(tb, 0)))
    for hp, base in ((1, 38), (2, 98), (3, 162)):
        for tb in range(TB):
            inj.append((base + tb, v_chunk(tb, hp)))
    for p, base in ((1, 34), (2, 94), (3, 158)):
        def mk_load(p=p):
            def emit():
                pair_w[p] = load_pair_weights(p)
            return once(("wld", p), lambda: [], emit)
        inj.append((base - 2, mk_load()))
        seq = [(1, 0), (0, 0), (0, 1), (1, 1), (1, 2), (1, 3)]
        for i, (qk, c) in enumerate(seq):
            qk_halves(p, qk, c, base + 3 * i)
        qk_halves(p, 0, 2, base + 24)
        qk_halves(p, 0, 3, base + 28)
    inj.sort(key=lambda t: t[0])
    inj = deque(inj)

    def pump_inj(i, budget=1):
        for _ in range(budget):
            src = None
            if inj and injl:
                src = inj if inj[0][0] <= injl[0][0] else injl
            elif inj or injl:
                src = inj or injl
            if src is None:
                return
            src.popleft()[1]()

    # ---- the main software-pipelined stream ----
    st2s = {}
    cur_iter = [0]

    def normalize(sc, h, acc):
        p, hl = h // 2, h % 2
        rec = recp.tile([128, 2, 4], F32, tag="rec", name=f"rec_{sc}_{h}")
        nc.vector.reciprocal(rec[:, 0, :], acc[0][:, :, DH:DH + 1])
        nc.vector.reciprocal(rec[:, 1, :], acc[1][:, :, DH:DH + 1])
        if hl == 0:
            st2s[(p, sc)] = st2p.tile([128, SB8, 128], F16, tag="st2",
                                      name=f"st2_{p}_{sc}")
        st2 = st2s[(p, sc)]
        for sb8 in range(SB8):
            nc.vector.tensor_scalar_mul(
                st2[:, sb8, hl * 64:(hl + 1) * 64],
                acc[sb8 // 4][:, sb8 % 4, 0:DH],
                rec[:, sb8 // 4, sb8 % 4:sb8 % 4 + 1])
        if hl == 1:
            # PE-transpose st2 into concatT, 4 s-blocks per psum tile
            for g in range(2):
                pt = ps_proj.tile([128, 512], F16, tag="proj",
                                  name=f"ct_{sc}_{p}_{g}")
                for j in range(4):
                    nc.tensor.transpose(pt[:, j * 128:(j + 1) * 128],
                                        st2[:, g * 4 + j, :], ident[:])
                nc.vector.tensor_copy(
                    concatT[:, p, sc * 1024 + g * 512:
                            sc * 1024 + (g + 1) * 512], pt[:])
            if p == 0:
                oacc[sc] = outp.tile([128, SB8, 2, 512], F16, tag="oacc",
                                     name=f"oacc_{sc}")
            for j, (sb8, ec) in enumerate((s8, e) for s8 in range(SB8)
                                          for e in range(2)):
                injl.append((cur_iter[0] + 1 + (3 * j) // 2,
                             outproj_partial(sc, sb8, ec, p)))

    iters = [(sc, h, tb) for (sc, h) in UNITS for tb in range(TB)]
    next_unit = {UNITS[i]: UNITS[i + 1] for i in range(len(UNITS) - 1)}
    pend = deque()
    accs = {}

    def acc_init(sc, h):
        def emit():
            acc = (ps_acc.tile([128, 4, DH + 1], F32, tag="acca",
                               name=f"acca_{sc}_{h}"),
                   ps_acc.tile([128, 4, DH + 1], F32, tag="accb",
                               name=f"accb_{sc}_{h}"))
            nc.vector.memset(acc[0][:], 0.0)
            nc.vector.memset(acc[1][:], 0.0)
            accs[(sc, h)] = acc
        return once(("acci", sc, h), lambda: [], emit)

    def flush_one():
        # psum accumulation "zero regions" are whole 2KB banks, so the 4
        # per-s-block groups in one bank can't use the start bit: zero the
        # banks once with the DVE, then accumulate with start=False.
        ex, sc, h, tb = pend.popleft()
        v_chunk(tb, h // 2)()
        if tb == 0:
            acc_init(sc, h)()
        acc = accs[(sc, h)]
        for sb8 in range(SB8):
            nc.tensor.matmul(acc[sb8 // 4][:, sb8 % 4, :],
                             ex[:, sb8 * 128:(sb8 + 1) * 128],
                             vext[:, tb, h, :],
                             start=False, stop=False, skip_group_check=True)
        if tb == TB - 1:
            normalize(sc, h, accs.pop((sc, h)))
            if (sc, h) in next_unit:
                acc_init(*next_unit[(sc, h)])()

    def emit_score(i):
        sc, h, tb = iters[i]
        hl, p = h % 2, h // 2
        if tb == 0:
            # everything this (sc, h) unit's scores need, emitted now if the
            # due-queue hasn't gotten to it yet
            if p not in pair_w:
                pair_w[p] = load_pair_weights(p)
                emitted.add(("wld", p))
            for c in (2 * sc, 2 * sc + 1):
                qk_half(p, 0, c, 0)(); qk_half(p, 0, c, 1)()
        kc = tb // 4
        qk_half(p, 1, kc, 0)(); qk_half(p, 1, kc, 1)()
        scp = ps_slot.tile([128, 1024], F32, tag="slot", name=f"s_{i}")
        kblk = kT[hl * 64:(hl + 1) * 64, p, tb * 128:(tb + 1) * 128]
        for half in range(2):
            c0 = sc * 1024 + half * 512
            nc.tensor.matmul(scp[:, half * 512:(half + 1) * 512], kblk,
                             qT[hl * 64:(hl + 1) * 64, p, c0:c0 + 512],
                             start=True, stop=True)
        return scp

    # scores are emitted one iteration ahead of their exp, so the PE reaches
    # the next score matmul the moment its pong slot frees instead of sitting
    # behind the attnV/chunk work of the current iteration
    scps = {0: emit_score(0)}
    for i, (sc, h, tb) in enumerate(iters):
        cur_iter[0] = i
        ex = exp_p.tile([128, 1024], F16, tag="ex", name=f"ex_{i}")
        nc.scalar.activation(ex[:], scps.pop(i)[:], Exp, scale=0.125)
        if i + 1 < len(iters):
            scps[i + 1] = emit_score(i + 1)
        pend.append((ex, sc, h, tb))
        # unit 0 runs scores/exp only (backlog 16) so the early iterations
        # have budget for the v/x/qk chunk stream; drain the backlog at half
        # rate (a double-flush iteration stalls the ACT) back to lag-1
        if i < 16:
            cap = 17
        else:
            cap = max(2, 17 - (i - 16) // 3)
        nflush = 0
        while pend and len(pend) > cap - 1 and nflush < 2:
            flush_one()
            nflush += 1
        pump_inj(i)

    while pend:
        flush_one()
    while inj or injl:
        pump_inj(1 << 30, budget=4)

    if dbg is not None:
        dbg_qt, dbg_kt, dbg_vx, dbg_cc, dbg_xt = dbg
        nc.sync.dma_start(dbg_qt[:], qT[:])
        nc.sync.dma_start(dbg_kt[:], kT[:])
        nc.sync.dma_start(dbg_vx[:], vext[:])
        nc.sync.dma_start(dbg_cc[:], concatT[:])
        nc.sync.dma_start(dbg_xt[:], xT[:])


_CACHE = {}


def _build(debug=False):
    nc = bacc.Bacc("TRN2", target_bir_lowering=False, debug=False,
                   num_devices=N_CORES)
    x_d = nc.dram_tensor("x", [S, E], F16, kind="ExternalInput").ap()
    wq_d = nc.dram_tensor("wq", [E, JW], F16, kind="ExternalInput").ap()
    wk_d = nc.dram_tensor("wk", [E, JW], F16, kind="ExternalInput").ap()
    wv_d = nc.dram_tensor("wv", [E, JW], F16, kind="ExternalInput").ap()
    wo_d = nc.dram_tensor("wo", [JW, E], F16, kind="ExternalInput").ap()
    bqt_d = nc.dram_tensor("bqt", [128, 4], F32, kind="ExternalInput").ap()
    bkt_d = nc.dram_tensor("bkt", [128, 4], F32, kind="ExternalInput").ap()
    bv_d = nc.dram_tensor("bv", [1, JW], F32, kind="ExternalInput").ap()
    id_d = nc.dram_tensor("ident", [128, 128], F16, kind="ExternalInput").ap()
    out_d = nc.dram_tensor("out", [S, E], F16, kind="ExternalOutput").ap()
    aps = (x_d, wq_d, wk_d, wv_d, wo_d, bqt_d, bkt_d, bv_d, id_d, out_d)
    dbg = None
    if debug:
        dbg = (
            nc.dram_tensor("dbg_qt", [128, 4, S], F16,
                           kind="ExternalOutput").ap(),
            nc.dram_tensor("dbg_kt", [128, 4, S], F16,
                           kind="ExternalOutput").ap(),
            nc.dram_tensor("dbg_vx", [128, TB, HPC, DH + 1], F16,
                           kind="ExternalOutput").ap(),
            nc.dram_tensor("dbg_cc", [128, 4, S], F16,
                           kind="ExternalOutput").ap(),
            nc.dram_tensor("dbg_xt", [128, EB, S], F16,
                           kind="ExternalOutput").ap(),
        )
    with tile.TileContext(nc) as tc:
        with ExitStack() as ctx:
            _emit(tc, aps, ctx, dbg=dbg)
    nc.compile()
    return nc


def kernel(x, Wq, bq, Wk, bk, Wv, bv, Wo, bo):
    x = np.asarray(x, dtype=np.float32)
    Wq = np.asarray(Wq, dtype=np.float32)
    bq = np.asarray(bq, dtype=np.float32)
    Wk = np.asarray(Wk, dtype=np.float32)
    bk = np.asarray(bk, dtype=np.float32)
    Wv = np.asarray(Wv, dtype=np.float32)
    bv = np.asarray(bv, dtype=np.float32)
    Wo = np.asarray(Wo, dtype=np.float32)
    bo = np.asarray(bo, dtype=np.float32)

    if "nc" not in _CACHE:
        _CACHE["nc"] = _build()
    nc = _CACHE["nc"]

    WoT = np.ascontiguousarray(Wo.T)  # [f, e]
    in_maps = []
    for c in range(N_CORES):
        b, hh = c // 2, c % 2
        hs = slice(hh * HPC, (hh + 1) * HPC)
        in_maps.append({
            "x": np.ascontiguousarray(x[b]).astype(np.float16),
            "wq": np.ascontiguousarray(
                Wq[hs].transpose(1, 0, 2).reshape(E, JW)).astype(np.float16),
            "wk": np.ascontiguousarray(
                Wk[hs].transpose(1, 0, 2).reshape(E, JW)).astype(np.float16),
            "wv": np.ascontiguousarray(
                Wv[hs].transpose(1, 0, 2).reshape(E, JW)).astype(np.float16),
            "wo": np.ascontiguousarray(
                WoT[hh * JW:(hh + 1) * JW]).astype(np.float16),
            "bqt": np.ascontiguousarray(bq[hs].reshape(4, 128).T),
            "bkt": np.ascontiguousarray(bk[hs].reshape(4, 128).T),
            "bv": bv[hs].reshape(1, JW),
            "ident": np.eye(128, dtype=np.float16),
        })

    res = bass_utils.run_bass_kernel_spmd(nc, in_maps,
                                          core_ids=list(range(N_CORES)))
    out = np.empty((B, S, E), dtype=np.float32)
    for b in range(B):
        out[b] = (res.results[2 * b]["out"].astype(np.float32)
                  + res.results[2 * b + 1]["out"].astype(np.float32))
        out[b] += bo[None, :]
    return out


# revision 66
# speedup vs baseline: 1.2312x; 1.0026x over previous
"""Multi-head attention kernel for Trainium2, 8 NeuronCores.

Problem (hardcoded shapes): B=4, S=2048, E=1024, H=16, DH=64.
  q/k/v = einsum('bse,hed->bhsd', x, W{q,k,v}) + b{q,k,v}
  attn  = softmax(q k^T / sqrt(DH)) v
  out   = concat_heads(attn) @ Wo^T + bo

Sharding: core c -> (batch b = c//2, head-half hh = c%2, i.e. heads
8*hh..8*hh+7).  Each core computes a [S, E] partial of its batch's output;
the host sums the two partials per batch and adds bo.

Everything 2-byte (fp16) on the PE so every matmul runs at 1 col/cycle:
  xT   [e=128 x 8, s=2048]  f16, PE-transpose of DMA'd x s-blocks
  qT/kT [j=128, pair, s]    f16, Wq^T x + bias per pair (PE + DVE bias)
  vext [t, tb, h, 65]       f16, x Wv + bias, with a fused ones column
  scores [t=128, s=1024]    PSUM f32, two 512-col matmuls per group
  exp   [t=128, s=1024]     one ACT Exp(scale=1/8) instr -> SBUF f16
  attnV FLIPPED: out[s=128, 65] += exp-block (lhsT) @ vext[t,65] (rhs);
        65-col moving operand -> half the PE cost of the [65, s] orientation.
        PSUM accumulation uses DVE-zeroed banks + start=False because the
        hardware "zero region" is a whole bank (4 groups share each bank).
  normalize: DVE recip on the sums column + per-s-block tensor_scalar
  concatT via PE-transpose of the normalized [s, f] blocks
  outproj: per-pair partial matmuls accumulated in SBUF f16 by the DVE, so
        only the last pair's 16 partials sit in the tail

Scheduling: one global software-pipelined stream over (s-chunk, head,
t-block) groups; projection/outproj work is broken into ~0.4-0.9us chunks
and injected one per iteration from a due-time queue.  Every consumer
*pulls* (emit-once) its producer chunks first, so emission order is correct
by construction regardless of the due tuning.

NOTE: dma_start_transpose is avoided entirely — its completion semaphore
fires before the data lands (verified empirically: a matmul chasing the
transpose reads garbage), and concurrent xbar transposes corrupt.
"""

import os
import sys

for _p in ("/opt/trn_rl_repo", "/root/.axon_site/_ro/trn_rl_repo"):
    if os.path.isdir(_p) and _p not in sys.path:
        sys.path.insert(0, _p)
        break

from collections import deque
from contextlib import ExitStack

import numpy as np

import concourse.bass as bass
import concourse.tile as tile
import concourse.mybir as mybir
from concourse import bacc, bass_utils

B, S, E, H, DH = 4, 2048, 1024, 16, 64
HPC = 8           # heads per core
JW = HPC * DH     # 512, per-core qkv width
N_CORES = 8
EB = E // 128     # 8 e-blocks
TB = S // 128     # 16 t-blocks (also x s-blocks)
SB8 = 8           # s-blocks per 1024-wide s-chunk
F32 = mybir.dt.float32
F16 = mybir.dt.float16
Exp = mybir.ActivationFunctionType.Exp
ADD = mybir.AluOpType.add

# (sc, h) processing order: interleave the two s-chunks so each qk pair's
# projection window is ~4 units wide and outproj partials spread out.
UNITS = []
for hp in range(4):
    UNITS += [(0, 2 * hp), (0, 2 * hp + 1), (1, 2 * hp), (1, 2 * hp + 1)]


def _emit(tc, aps, ctx, dbg=None):
    nc = tc.nc
    x_d, wq_d, wk_d, wv_d, wo_d, bqt_d, bkt_d, bv_d, id_d, out_d = aps

    def pool(**kw):
        return ctx.enter_context(tc.tile_pool(**kw))

    const = pool(name="const", bufs=1)
    xs = pool(name="xs", bufs=1)
    xTp = pool(name="xT", bufs=1)
    vxp = pool(name="vext", bufs=1)
    wqk = pool(name="wqk", bufs=2)
    qkp = pool(name="qk", bufs=1)
    exp_p = pool(name="expS", bufs=17)
    st2p = pool(name="st2", bufs=2)
    recp = pool(name="rec", bufs=2)
    ccp = pool(name="concatT", bufs=1)
    outp = pool(name="outs", bufs=2)
    ps_slot = pool(name="ps_slot", bufs=2, space="PSUM")
    ps_acc = pool(name="ps_acc", bufs=1, space="PSUM")
    ps_proj = pool(name="ps_proj", bufs=1, space="PSUM")

    # ---- DMAs, in consumption order, all on the sync queue ----
    wq_r = wq_d.rearrange("(eb pp) j -> pp eb j", pp=128)
    wk_r = wk_d.rearrange("(eb pp) j -> pp eb j", pp=128)

    def load_pair_weights(p):
        wk_t = wqk.tile([128, EB, 128], F16, tag="wk", name=f"wk{p}")
        nc.sync.dma_start(wk_t[:], wk_r[:, :, p * 128:(p + 1) * 128])
        wq_t = wqk.tile([128, EB, 128], F16, tag="wq", name=f"wq{p}")
        nc.sync.dma_start(wq_t[:], wq_r[:, :, p * 128:(p + 1) * 128])
        return wq_t, wk_t

    wv_sb = const.tile([128, EB, JW], F16)
    wo_sb = const.tile([128, 4, E], F16)

    # x staged in four 4-s-block group DMAs (one instruction each), with the
    # weight loads slotted between them in consumption order
    x_groups = {}
    x_r = x_d.rearrange("(g p) e -> p g e", p=128)

    def dma_xg(g):
        x_t = xs.tile([128, 4, E], F16, tag="x_t", name=f"x_g_{g}")
        nc.sync.dma_start(x_t[:], x_r[:, 4 * g:4 * g + 4, :])
        x_groups[g] = x_t

    dma_xg(0)
    ident = const.tile([128, 128], F16)
    nc.sync.dma_start(ident[:], id_d[:])
    bq_sb = const.tile([128, 4], F32)
    nc.sync.dma_start(bq_sb[:], bqt_d[:])
    bk_sb = const.tile([128, 4], F32)
    nc.sync.dma_start(bk_sb[:], bkt_d[:])
    bv1 = const.tile([1, JW], F32)
    nc.sync.dma_start(bv1[:], bv_d[:])
    pair_w = {0: load_pair_weights(0)}
    dma_xg(1)
    dma_xg(2)
    dma_xg(3)
    nc.sync.dma_start(wv_sb[:], wv_d.rearrange("(eb p) j -> p eb j", p=128))
    nc.sync.dma_start(wo_sb[:], wo_d.rearrange("(fb p) e -> p fb e", p=128))

    # PE p-state warmup: the tensor engine ramps to full clock only after
    # ~3us of sustained use, and the ramp clock starts at the first busy
    # period.  Run throwaway matmuls from t~0.3us so the real prefix work
    # (from ~4.5us, when x group 0 lands) runs at full speed.
    warm = const.tile([128, 512], F16)
    nc.vector.memset(warm[:], 0.25)
    for i in range(14):
        pw = ps_slot.tile([128, 512], F32, tag="slot", name=f"warm{i}")
        nc.tensor.matmul(pw[:], warm[:, 0:128], warm[:], start=True, stop=True)

    bvb = const.tile([128, JW], F32)
    nc.gpsimd.partition_broadcast(bvb[:], bv1[:])

    vext = vxp.tile([128, TB, HPC, DH + 1], F16)
    nc.gpsimd.memset(vext[:, :, :, DH:DH + 1], 1.0)

    qT = qkp.tile([128, 4, S], F16, tag="qT")
    kT = qkp.tile([128, 4, S], F16, tag="kT")
    xT = xTp.tile([128, EB, S], F16)
    concatT = ccp.tile([128, 4, S], F16)

    # ---- emit-once chunk machinery: consumers pull producers ----
    emitted = set()

    def once(key, deps, fn):
        def run():
            if key in emitted:
                return
            emitted.add(key)
            for d in deps():
                d()
            fn()
        return run

    def x_chunk(sb):
        """PE-transpose x s-block sb into xT.  Uses the score-slot psum pool
        (2 buffers, idle during the prefix) so consecutive chunks double-
        buffer instead of serializing on a single-bank WAR chain."""
        def emit():
            x_t = x_groups[sb // 4][:, sb % 4, :]
            pt = ps_slot.tile([128, 1024], F16, tag="slot", name=f"pt_{sb}")
            for eb in range(EB):
                nc.tensor.transpose(pt[:, eb * 128:(eb + 1) * 128],
                                    x_t[:, eb * 128:(eb + 1) * 128],
                                    ident[:])
            nc.vector.tensor_copy(
                xT[:, :, sb * 128:(sb + 1) * 128],
                pt[:].rearrange("p (e s) -> p e s", e=8))
        return once(("x", sb), lambda: [], emit)

    _qk_ps = {}
    _qk_open = [None]
    _qk_use_slot = [True]  # prefix chunks double-buffer via the slot pool

    def qk_half(p, qk, c, half):
        """Half (4 e-blocks) of one 512-col q/k projection chunk.  The "pq"
        psum pool has one buffer, so before opening a new chunk any other
        half-open chunk is closed first."""
        def emit():
            if half == 0 and _qk_open[0] is not None:
                op, oqk, oc = _qk_open[0]
                qk_half(op, oqk, oc, 1)()
            wq_t, wk_t = pair_w[p]
            w_t, dst, b_sb = ((wq_t, qT, bq_sb) if qk == 0 else
                              (wk_t, kT, bk_sb))
            if half == 0:
                if _qk_use_slot[0]:
                    pq = ps_slot.tile([128, 512], F32, tag="slot",
                                      name=f"pq_{p}_{qk}_{c}")
                else:
                    pq = ps_proj.tile([128, 512], F32, tag="pq",
                                      name=f"pq_{p}_{qk}_{c}")
                    _qk_open[0] = (p, qk, c)
                _qk_ps[(p, qk, c)] = pq
            else:
                pq = _qk_ps.pop((p, qk, c))
                if not _qk_use_slot[0]:
                    _qk_open[0] = None
            for q in range(4):
                eb = half * 4 + q
                nc.tensor.matmul(pq[:], w_t[:, eb, :],
                                 xT[:, eb, c * 512:(c + 1) * 512],
                                 start=(eb == 0), stop=(eb == EB - 1))
            if half == 1:
                nc.vector.tensor_scalar_add(
                    dst[:, p, c * 512:(c + 1) * 512], pq[:], b_sb[:, p:p + 1])

        def deps():
            d = [x_chunk(sb) for sb in range(4 * c, 4 * c + 4)]
            if half == 1:
                d.append(qk_half(p, qk, c, 0))
            return d
        return once(("qk", p, qk, c, half), deps, emit)

    def v_chunk(tb, hp):
        """Project v for t-block tb, head pair hp (2 heads, 128 cols).
        Alternates between the proj and (when no q/k chunk is half-open)
        pq psum banks so consecutive chunks double-buffer."""
        def emit():
            pv = ps_proj.tile([128, 128], F32, tag="proj",
                              name=f"pv_{tb}_{hp}")
            for eb in range(EB):
                nc.tensor.matmul(pv[:], xT[:, eb, tb * 128:(tb + 1) * 128],
                                 wv_sb[:, eb, hp * 128:(hp + 1) * 128],
                                 start=(eb == 0), stop=(eb == EB - 1))
            nc.vector.tensor_tensor(
                vext[:, tb, 2 * hp:2 * hp + 2, 0:DH],
                pv[:].rearrange("p (h d) -> p h d", h=2),
                bvb[:, hp * 128:(hp + 1) * 128].rearrange(
                    "p (h d) -> p h d", h=2), ADD)
        return once(("v", tb, hp), lambda: [x_chunk(tb)], emit)

    oacc = {}

    def outproj_partial(sc, sb8, ec, p):
        def emit():
            # the last pair of the last s-chunk is the kernel tail: ping-pong
            # through the freed "pq" bank, and route half the accumulates
            # through the idle ACT engine (PE identity-matmul adds oacc into
            # the psum group, ACT copies it out) so DVE and ACT split the work
            tail = (p == 3 and sc == 1)
            tag = "pq" if (tail and (sb8 + ec) % 2) else "proj"
            po = ps_proj.tile([128, 512], F32, tag=tag,
                              name=f"po_{sc}_{sb8}_{ec}_{p}")
            oa = oacc[sc][:, sb8, ec, :]
            act_lane = tail and (sb8 + ec) % 2
            nc.tensor.matmul(po[:],
                             concatT[:, p, sc * 1024 + sb8 * 128:
                                     sc * 1024 + (sb8 + 1) * 128],
                             wo_sb[:, p, ec * 512:(ec + 1) * 512],
                             start=True, stop=not act_lane)
            if act_lane:
                nc.tensor.matmul(po[:], ident[:], oa, start=False, stop=True)
                nc.scalar.copy(oa, po[:])
            elif p == 0:
                nc.vector.tensor_copy(oa, po[:])
            else:
                nc.vector.tensor_tensor(oa, po[:], oa, ADD)
            if p == 3 and ec == 1:
                r0 = sc * 1024 + sb8 * 128
                nc.sync.dma_start(out_d[r0:r0 + 128, :],
                                  oacc[sc][:, sb8, :, :])
        return once(("op", sc, sb8, ec, p), lambda: [], emit)

    # ---- prefix PE work: x transposes chase the x DMAs; pair-0 k/q chunks
    # chase the transposes e-block by e-block ----
    for sb in range(4):
        x_chunk(sb)()
    qk_half(0, 1, 0, 0)(); qk_half(0, 1, 0, 1)()
    qk_half(0, 0, 0, 0)(); qk_half(0, 0, 0, 1)()
    for sb in range(4, 8):
        x_chunk(sb)()
    qk_half(0, 0, 1, 0)(); qk_half(0, 0, 1, 1)()
    _qk_use_slot[0] = False

    # ---- due-scheduled injection queue (performance tuning only) ----
    inj = []
    injl = deque()

    def qk_halves(p, qk, c, due):
        inj.append((due, qk_half(p, qk, c, 0)))
        inj.append((due, qk_half(p, qk, c, 1)))

    for sb in range(8, TB):
        inj.append((sb - 4, x_chunk(sb)))
    qk_halves(0, 1, 1, 2)
    qk_halves(0, 1, 2, 8)
    qk_halves(0, 1, 3, 12)
    qk_halves(0, 0, 2, 17)
    qk_halves(0, 0, 3, 21)
    # unit 0's attnV is deferred ~16 iters, so vext t-blocks are only needed
    # from iter ~16+tb
    for tb in range(1, TB):
        inj.append((11 + tb, v_chunk(tb, 0)))
    for hp, base in ((1, 38), (2, 98), (3, 162)):
        for tb in range(TB):
            inj.append((base + tb, v_chunk(tb, hp)))
    for p, base in ((1, 34), (2, 94), (3, 158)):
        def mk_load(p=p):
            def emit():
                pair_w[p] = load_pair_weights(p)
            return once(("wld", p), lambda: [], emit)
        inj.append((base - 2, mk_load()))
        seq = [(1, 0), (0, 0), (0, 1), (1, 1), (1, 2), (1, 3)]
        for i, (qk, c) in enumerate(seq):
            qk_halves(p, qk, c, base + 3 * i)
        qk_halves(p, 0, 2, base + 24)
        qk_halves(p, 0, 3, base + 28)
    inj.sort(key=lambda t: t[0])
    inj = deque(inj)

    def pump_inj(i, budget=1):
        for _ in range(budget):
            src = None
            if inj and injl:
                src = inj if inj[0][0] <= injl[0][0] else injl
            elif inj or injl:
                src = inj or injl
            if src is None:
                return
            src.popleft()[1]()

    # ---- the main software-pipelined stream ----
    st2s = {}
    cur_iter = [0]

    def normalize(sc, h, acc):
        p, hl = h // 2, h % 2
        rec = recp.tile([128, 2, 4], F32, tag="rec", name=f"rec_{sc}_{h}")
        nc.vector.reciprocal(rec[:, 0, :], acc[0][:, :, DH:DH + 1])
        nc.vector.reciprocal(rec[:, 1, :], acc[1][:, :, DH:DH + 1])
        if hl == 0:
            st2s[(p, sc)] = st2p.tile([128, SB8, 128], F16, tag="st2",
                                      name=f"st2_{p}_{sc}")
        st2 = st2s[(p, sc)]
        for sb8 in range(SB8):
            nc.vector.tensor_scalar_mul(
                st2[:, sb8, hl * 64:(hl + 1) * 64],
                acc[sb8 // 4][:, sb8 % 4, 0:DH],
                rec[:, sb8 // 4, sb8 % 4:sb8 % 4 + 1])
        if hl == 1:
            # PE-transpose st2 into concatT, 4 s-blocks per psum tile
            for g in range(2):
                pt = ps_proj.tile([128, 512], F16, tag="proj",
                                  name=f"ct_{sc}_{p}_{g}")
                for j in range(4):
                    nc.tensor.transpose(pt[:, j * 128:(j + 1) * 128],
                                        st2[:, g * 4 + j, :], ident[:])
                nc.vector.tensor_copy(
                    concatT[:, p, sc * 1024 + g * 512:
                            sc * 1024 + (g + 1) * 512], pt[:])
            if p == 0:
                oacc[sc] = outp.tile([128, SB8, 2, 512], F16, tag="oacc",
                                     name=f"oacc_{sc}")
            for j, (sb8, ec) in enumerate((s8, e) for s8 in range(SB8)
                                          for e in range(2)):
                injl.append((cur_iter[0] + 1 + (3 * j) // 2,
                             outproj_partial(sc, sb8, ec, p)))

    iters = [(sc, h, tb) for (sc, h) in UNITS for tb in range(TB)]
    next_unit = {UNITS[i]: UNITS[i + 1] for i in range(len(UNITS) - 1)}
    pend = deque()
    accs = {}

    def acc_init(sc, h):
        def emit():
            acc = (ps_acc.tile([128, 4, DH + 1], F32, tag="acca",
                               name=f"acca_{sc}_{h}"),
                   ps_acc.tile([128, 4, DH + 1], F32, tag="accb",
                               name=f"accb_{sc}_{h}"))
            nc.vector.memset(acc[0][:], 0.0)
            nc.vector.memset(acc[1][:], 0.0)
            accs[(sc, h)] = acc
        return once(("acci", sc, h), lambda: [], emit)

    def flush_one():
        # psum accumulation "zero regions" are whole 2KB banks, so the 4
        # per-s-block groups in one bank can't use the start bit: zero the
        # banks once with the DVE, then accumulate with start=False.
        ex, sc, h, tb = pend.popleft()
        v_chunk(tb, h // 2)()
        if tb == 0:
            acc_init(sc, h)()
        acc = accs[(sc, h)]
        for sb8 in range(SB8):
            nc.tensor.matmul(acc[sb8 // 4][:, sb8 % 4, :],
                             ex[:, sb8 * 128:(sb8 + 1) * 128],
                             vext[:, tb, h, :],
                             start=False, stop=False, skip_group_check=True)
        if tb == TB - 1:
            normalize(sc, h, accs.pop((sc, h)))
            if (sc, h) in next_unit:
                acc_init(*next_unit[(sc, h)])()

    def emit_score(i):
        sc, h, tb = iters[i]
        hl, p = h % 2, h // 2
        if tb == 0:
            # everything this (sc, h) unit's scores need, emitted now if the
            # due-queue hasn't gotten to it yet
            if p not in pair_w:
                pair_w[p] = load_pair_weights(p)
                emitted.add(("wld", p))
            for c in (2 * sc, 2 * sc + 1):
                qk_half(p, 0, c, 0)(); qk_half(p, 0, c, 1)()
        kc = tb // 4
        qk_half(p, 1, kc, 0)(); qk_half(p, 1, kc, 1)()
        scp = ps_slot.tile([128, 1024], F32, tag="slot", name=f"s_{i}")
        kblk = kT[hl * 64:(hl + 1) * 64, p, tb * 128:(tb + 1) * 128]
        for half in range(2):
            c0 = sc * 1024 + half * 512
            nc.tensor.matmul(scp[:, half * 512:(half + 1) * 512], kblk,
                             qT[hl * 64:(hl + 1) * 64, p, c0:c0 + 512],
                             start=True, stop=True)
        return scp

    # scores are emitted one iteration ahead of their exp, so the PE reaches
    # the next score matmul the moment its pong slot frees instead of sitting
    # behind the attnV/chunk work of the current iteration
    scps = {0: emit_score(0)}
    for i, (sc, h, tb) in enumerate(iters):
        cur_iter[0] = i
        ex = exp_p.tile([128, 1024], F16, tag="ex", name=f"ex_{i}")
        nc.scalar.activation(ex[:], scps.pop(i)[:], Exp, scale=0.125)
        if i + 1 < len(iters):
            scps[i + 1] = emit_score(i + 1)
        pend.append((ex, sc, h, tb))
        # unit 0 runs scores/exp only (backlog 16) so the early iterations
        # have budget for the v/x/qk chunk stream; drain the backlog at half
        # rate (a double-flush iteration stalls the ACT) back to lag-1
        if i < 16:
            cap = 17
        else:
            cap = max(2, 17 - (i - 16) // 3)
        nflush = 0
        while pend and len(pend) > cap - 1 and nflush < 2:
            flush_one()
            nflush += 1
        pump_inj(i)

    while pend:
        flush_one()
    while inj or injl:
        pump_inj(1 << 30, budget=4)

    if dbg is not None:
        dbg_qt, dbg_kt, dbg_vx, dbg_cc, dbg_xt = dbg
        nc.sync.dma_start(dbg_qt[:], qT[:])
        nc.sync.dma_start(dbg_kt[:], kT[:])
        nc.sync.dma_start(dbg_vx[:], vext[:])
        nc.sync.dma_start(dbg_cc[:], concatT[:])
        nc.sync.dma_start(dbg_xt[:], xT[:])


_CACHE = {}


def _build(debug=False):
    nc = bacc.Bacc("TRN2", target_bir_lowering=False, debug=False,
                   num_devices=N_CORES)
    x_d = nc.dram_tensor("x", [S, E], F16, kind="ExternalInput").ap()
    wq_d = nc.dram_tensor("wq", [E, JW], F16, kind="ExternalInput").ap()
    wk_d = nc.dram_tensor("wk", [E, JW], F16, kind="ExternalInput").ap()
    wv_d = nc.dram_tensor("wv", [E, JW], F16, kind="ExternalInput").ap()
    wo_d = nc.dram_tensor("wo", [JW, E], F16, kind="ExternalInput").ap()
    bqt_d = nc.dram_tensor("bqt", [128, 4], F32, kind="ExternalInput").ap()
    bkt_d = nc.dram_tensor("bkt", [128, 4], F32, kind="ExternalInput").ap()
    bv_d = nc.dram_tensor("bv", [1, JW], F32, kind="ExternalInput").ap()
    id_d = nc.dram_tensor("ident", [128, 128], F16, kind="ExternalInput").ap()
    out_d = nc.dram_tensor("out", [S, E], F16, kind="ExternalOutput").ap()
    aps = (x_d, wq_d, wk_d, wv_d, wo_d, bqt_d, bkt_d, bv_d, id_d, out_d)
    dbg = None
    if debug:
        dbg = (
            nc.dram_tensor("dbg_qt", [128, 4, S], F16,
                           kind="ExternalOutput").ap(),
            nc.dram_tensor("dbg_kt", [128, 4, S], F16,
                           kind="ExternalOutput").ap(),
            nc.dram_tensor("dbg_vx", [128, TB, HPC, DH + 1], F16,
                           kind="ExternalOutput").ap(),
            nc.dram_tensor("dbg_cc", [128, 4, S], F16,
                           kind="ExternalOutput").ap(),
            nc.dram_tensor("dbg_xt", [128, EB, S], F16,
                           kind="ExternalOutput").ap(),
        )
    with tile.TileContext(nc) as tc:
        with ExitStack() as ctx:
            _emit(tc, aps, ctx, dbg=dbg)
    nc.compile()
    return nc


def kernel(x, Wq, bq, Wk, bk, Wv, bv, Wo, bo):
    x = np.asarray(x, dtype=np.float32)
    Wq = np.asarray(Wq, dtype=np.float32)
    bq = np.asarray(bq, dtype=np.float32)
    Wk = np.asarray(Wk, dtype=np.float32)
    bk = np.asarray(bk, dtype=np.float32)
    Wv = np.asarray(Wv, dtype=np.float32)
    bv = np.asarray(bv, dtype=np.float32)
    Wo = np.asarray(Wo, dtype=np.float32)
    bo = np.asarray(bo, dtype=np.float32)

    if "nc" not in _CACHE:
        _CACHE["nc"] = _build()
    nc = _CACHE["nc"]

    WoT = np.ascontiguousarray(Wo.T)  # [f, e]
    in_maps = []
    for c in range(N_CORES):
        b, hh = c // 2, c % 2
        hs = slice(hh * HPC, (hh + 1) * HPC)
        in_maps.append({
            "x": np.ascontiguousarray(x[b]).astype(np.float16),
            "wq": np.ascontiguousarray(
                Wq[hs].transpose(1, 0, 2).reshape(E, JW)).astype(np.float16),
            "wk": np.ascontiguousarray(
                Wk[hs].transpose(1, 0, 2).reshape(E, JW)).astype(np.float16),
            "wv": np.ascontiguousarray(
                Wv[hs].transpose(1, 0, 2).reshape(E, JW)).astype(np.float16),
            "wo": np.ascontiguousarray(
                WoT[hh * JW:(hh + 1) * JW]).astype(np.float16),
            "bqt": np.ascontiguousarray(bq[hs].reshape(4, 128).T),
            "bkt": np.ascontiguousarray(bk[hs].reshape(4, 128).T),
            "bv": bv[hs].reshape(1, JW),
            "ident": np.eye(128, dtype=np.float16),
        })

    res = bass_utils.run_bass_kernel_spmd(nc, in_maps,
                                          core_ids=list(range(N_CORES)))
    out = np.empty((B, S, E), dtype=np.float32)
    for b in range(B):
        out[b] = (res.results[2 * b]["out"].astype(np.float32)
                  + res.results[2 * b + 1]["out"].astype(np.float32))
        out[b] += bo[None, :]
    return out
